# revision 43
# baseline (speedup 1.0000x reference)
"""Trainium2 Bass kernel for nn_MirasModel (scatter_memory).

Strategy (8 NeuronCores, SPMD):
  - Column-shard the shared D=3136 feature dimension: core c owns Dc=392
    columns of dense_k_w / dense_v_w / mem_w2 / biases / scales, and the
    matching 392 rows of mem_w1.
  - Conv + rmsnorm computed fully on every core (tiny) via a packed
    im2col matmul, with a DMA scatter producing the transposed
    [Din, T] activation layout the dense matmuls need.
  - Three AllReduce rounds:
      R1: z1 = keys@w1+b1 partial sums  +  Gram(keys) = keys keys^T
      R2: per-token scalars (C,A,B) + backward projections P1,P2,P3
      R3: final-forward rmsnorm scalar partials
    The Gram matrix lets z1f = z1 - G_K diag(w) dz1 be computed locally,
    eliminating a fourth round (keys @ agg_w1 == Gram @ diag(w) @ dz1).
  - All heavy DMA (im2col + dense weight shards) hides under R1's
    collective entry latency.
"""

import sys

if '/opt/trn_rl_repo' not in sys.path:
    sys.path.insert(0, '/opt/trn_rl_repo')

import numpy as np

import concourse.bass as bass
import concourse.mybir as mybir
from concourse import tile
from concourse.bass_utils import run_bass_kernel_spmd

F32 = mybir.dt.float32
F32R = mybir.dt.float32r
BF16 = mybir.dt.bfloat16
NPBF = mybir.dt.np(mybir.dt.bfloat16)
AF = mybir.ActivationFunctionType
OP = mybir.AluOpType

T = 64
D = 3136
H = 512
NCORES = 8
DC = D // NCORES            # 392 columns per core
CQ = 98                     # Dc sub-chunk (4 per core)
NQ = DC // CQ               # 4
PPIX = 800                  # padded pixel count (784 real + 16 dummy)
DINP = PPIX * 4             # padded Din = 3200
RT = DINP // 128            # 25 Din tiles
NPTR = PPIX // 2            # 400 pixel-pairs
NCONV = NPTR * T // 512     # 50 conv matmul chunks
CPAIR = 10                  # conv chunks per DMA slab (one r-group)
NSLAB = NCONV // CPAIR      # 25 slabs
XROWS = 80                  # 73 im2col rows padded to 80 (16 | 80*512)
HT = H // 128               # 4 H tiles
ALPHA, ETA0, EPS = 0.9, 0.1, 1e-6

_NC_CACHE = {}


# ---------------------------------------------------------------------------
# walrus workaround: this compiler build rejects Drain instructions carrying
# more than one sync wait; split extras onto preceding Drains.
def _split_excess_waits(nc):
    """This walrus build has tight per-instruction sync-wait budgets
    (1 for Drain/Matmult/etc).  Move excess waits onto preceding NoOps."""
    LIM1 = 1

    def limit_for(ins):
        return LIM1

    n_new = 0
    for fn in nc.m.functions:
        for bb in fn.blocks:
            i = 0
            while i < len(bb.instructions):
                ins = bb.instructions[i]
                si = getattr(ins, 'sync_info', None)
                lim = limit_for(ins)
                if (si is not None and si.on_wait and len(si.on_wait) > lim
                        and getattr(ins, 'engine', None) is not None):
                    waits = list(si.on_wait)
                    keep, extra = waits[:lim], waits[lim:]
                    ins.sync_info = mybir.SyncInfo(on_wait=keep,
                                                  on_update=si.on_update)
                    pos = i
                    for j in range(0, len(extra), LIM1):
                        n_new += 1
                        nd = mybir.InstNoOp(
                            name=f"I-waitfix-{n_new}",
                            engine=ins.engine,
                            bass_nofuse=True,
                            sync_info=mybir.SyncInfo(
                                on_wait=extra[j:j + LIM1], on_update=[]),
                        )
                        bb.instructions.insert(pos, nd)
                        pos += 1
                        i += 1
                i += 1
    return n_new


def _din_perm():
    """Device Din row -> reference Din index (p*4+c), p,c of padded grid."""
    idx = np.zeros(DINP, np.int64)
    valid = np.zeros(DINP, bool)
    for r in range(RT):
        for i in range(128):
            g, c, jj = i // 64, (i % 64) // 16, i % 16
            p = 2 * (16 * r + jj) + g
            row = r * 128 + i
            if p < 784:
                idx[row] = p * 4 + c
                valid[row] = True
    return idx, valid


def _pack_slabs(X72):
    """Repack [73, NCONV*512] im2col into DMA-slab layout.

    Returns [NSLAB*XROWS, CPAIR*512]; slab s rows 0:73 = X72 cols
    s*CPAIR*512:(s+1)*CPAIR*512, rows 73:80 zero.  Row-contiguous slabs
    let the HWDGE spray descriptors across all 16 DMA engines (a
    strided source pins the whole transfer to one engine).
    """
    Xs = np.zeros((NSLAB * XROWS, CPAIR * 512), np.float32)
    v = X72.reshape(73, NSLAB, CPAIR * 512)
    for s in range(NSLAB):
        Xs[s * XROWS:s * XROWS + 73, :] = v[:, s, :]
    return Xs


def _build_im2col(x_t, pad_val=0.0):
    """x_t: (T, 28, 28, 4) NHWC.  Returns X72 [73, NPTR*64] fp32.

    row = g*36 + (di*3+dj)*4 + ci  (g in 0..1), row 72 = ones.
    col = ptr*64 + t, pixel p = 2*ptr + g (row-major 28x28, padded to 800).
    """
    xp = np.zeros((T, 30, 30, 4), np.float32)
    xp[:, 1:29, 1:29, :] = x_t
    X = np.zeros((73, NPTR * T), np.float32)
    p = np.arange(PPIX)
    pi, pj = p // 28, p % 28
    ok = p < 784
    for g in range(2):
        psel = p[(p % 2) == g]
        ptr = psel // 2
        pis, pjs, oks = pi[(p % 2) == g], pj[(p % 2) == g], ok[(p % 2) == g]
        for di in range(3):
            for dj in range(3):
                for ci in range(4):
                    row = g * 36 + (di * 3 + dj) * 4 + ci
                    vals = np.zeros((NPTR, T), np.float32)
                    vsel = xp[:, np.clip(pis + di, 0, 29),
                              np.clip(pjs + dj, 0, 29), ci]  # (T, NPTR)
                    vals[oks[: NPTR], :] = vsel.T[oks[: NPTR], :]
                    # dummy pixels (>=784) contribute garbage later discarded
                    X[row, :] = vals.reshape(-1)
    X[72, :] = 1.0
    return X


def _build_w73(conv_k_w, conv_k_b, conv_v_w, conv_v_b):
    """W73 [73, 16]; col = g*8 + kv*4 + co."""
    W = np.zeros((73, 16), np.float32)
    for g in range(2):
        for kv, (w, b) in enumerate(((conv_k_w, conv_k_b),
                                     (conv_v_w, conv_v_b))):
            for di in range(3):
                for dj in range(3):
                    for ci in range(4):
                        W[g * 36 + (di * 3 + dj) * 4 + ci,
                          g * 8 + kv * 4:g * 8 + kv * 4 + 4] = w[di, dj, ci, :]
            W[72, g * 8 + kv * 4:g * 8 + kv * 4 + 4] = b
    return W


def _rms_pattern(scale4):
    """[128,1] per-partition rms scale: partition i -> scale4[(i%64)//16]."""
    i = np.arange(128)
    return scale4[(i % 64) // 16].astype(np.float32).reshape(128, 1)


def _s4():
    """Dup-selector [128, 128]: S[i, o] = 1 iff (g, j) of i == (g, j) of o.

    Partition layout (g, c, j): g = i // 64, c = (i % 64) // 16, j = i % 16.
    The sumsq matmul with this stationary yields the per-pixel channel
    sum-of-squares already duplicated across the 4 c-slots."""
    i = np.arange(128)
    gj = (i // 64) * 16 + (i % 16)
    return (gj[:, None] == gj[None, :]).astype(np.float32)


def _wvec():
    betas = (np.float32(ALPHA) ** np.arange(T, dtype=np.float32)).astype(np.float32)
    etas = (np.float32(ETA0) * betas).astype(np.float32)
    weights = (etas * (betas[-1] / betas)).astype(np.float32)
    return (np.float32(1e-4) * weights).astype(np.float32)


def build_nc(debug=False):
    nc = bass.Bass()

    def inp(name, shape, dt=F32):
        return nc.dram_tensor(name, list(shape), dt, kind="ExternalInput")

    X80 = inp('X80', (NSLAB * XROWS, CPAIR * 512), BF16)
    W73 = inp('W73', (73, 16), BF16)
    WkC = inp('WkC', (128, RT * DC), BF16)   # p-major repack of [DINP, DC]
    WvC = inp('WvC', (128, RT * DC), BF16)
    bkC = inp('bkC', (1, DC), BF16)
    bvC = inp('bvC', (1, DC), BF16)
    w1C = inp('w1C', (CQ, NQ * H), BF16)  # w1 rows chunked: [:, q*H+h]
    b1row8 = inp('b1row8', (1, H), BF16)  # mem_b1 / 8
    w2C = inp('w2C', (128, HT * DC), BF16)  # w2 H-chunked: [:, m*DC+d]
    b2C = inp('b2C', (1, DC))
    scC = inp('scC', (1, DC), BF16)
    scC32 = inp('scC32', (1, DC))
    rosC = inp('rosC', (1, DC))
    scsqT = inp('scsqT', (CQ, NQ))        # mem_scale[sl]**2, chunked columns
    rmspk = inp('rmspk', (128, 1))
    rmspv = inp('rmspv', (128, 1))
    S4 = inp('S4', (128, 128), BF16)   # dup-selector
    wv = inp('wv', (T, 1))                # 1e-4 * weights
    ones1x64 = inp('ones1x64', (1, T), BF16)
    ones1x128 = inp('ones1x128', (1, 128))
    onescol = inp('onescol', (128, 1), BF16)
    ident = inp('ident', (128, 128))
    identb = inp('identb', (128, 128), BF16)

    out = nc.dram_tensor('out', [CQ, NQ * T], F32, kind="ExternalOutput")
    dbg_outs = {}

    def dbg(name, shape):
        if debug:
            dbg_outs[name] = nc.dram_tensor(name, list(shape), F32,
                                            kind="ExternalOutput")
        return dbg_outs.get(name)

    d_nkT = dbg('d_nkT', (128, RT * T))
    d_keys = dbg('d_keys', (T, DC))
    d_vals = dbg('d_vals', (T, DC))
    d_z1T = dbg('d_z1T', (H, T))
    d_GK = dbg('d_GK', (T, T))
    d_y = dbg('d_y', (T, DC))
    d_P = dbg('d_P', (3 * H, T))
    d_dhT = dbg('d_dhT', (H, T))
    d_z1fT = dbg('d_z1fT', (H, T))
    d_w2p = dbg('d_w2p', (H, DC))
    d_yfT = dbg('d_yfT', (DC, T))

    with tile.TileContext(nc) as tc:
        with (
            tc.tile_pool(name='consts', bufs=1) as pc,
            tc.tile_pool(name='wstream', bufs=4) as pw,
            tc.tile_pool(name='xstream', bufs=4) as px,
            tc.tile_pool(name='big', bufs=1) as pb,
            tc.tile_pool(name='work', bufs=1) as pk,
            tc.tile_pool(name='psA', bufs=2, space='PSUM') as psA,
            tc.tile_pool(name='psB', bufs=2, space='PSUM') as psB,
            tc.tile_pool(name='psT', bufs=2, space='PSUM') as psT,
            tc.tile_pool(name='dram', bufs=1, space='DRAM') as pd,
        ):
            # ---- small constants to SBUF ----
            # sync queue: only what conv needs, so X80 slabs aren't stuck
            # behind the 5 MB weight preloads; everything else rides the
            # scalar HWDGE queue (small tiles first).
            def lc(ap, shape, name, dt=F32, eng=None):
                t_ = pc.tile(list(shape), dt, name=name)
                (eng or nc.sync).dma_start(t_[:], ap[:])
                return t_

            W73s = lc(W73, (73, 16), 'W73s', BF16)
            rpk = lc(rmspk, (128, 1), 'rpk')
            rpv = lc(rmspv, (128, 1), 'rpv')
            S4s = lc(S4, (128, 128), 'S4s', BF16)
            sc_ = nc.scalar
            bkS = lc(bkC, (1, DC), 'bkS', BF16, sc_)
            bvS = lc(bvC, (1, DC), 'bvS', BF16, sc_)
            b1r8 = lc(b1row8, (1, H), 'b1r8', BF16, sc_)
            b2S = lc(b2C, (1, DC), 'b2S', F32, sc_)
            scS = lc(scC, (1, DC), 'scS', BF16, sc_)
            scS32 = lc(scC32, (1, DC), 'scS32', F32, sc_)
            rosS = lc(rosC, (1, DC), 'rosS', F32, sc_)
            scsqTS = lc(scsqT, (CQ, NQ), 'scsqTS', F32, sc_)
            wvS = lc(wv, (T, 1), 'wvS', F32, sc_)
            o64 = lc(ones1x64, (1, T), 'o64', BF16, sc_)
            o128 = lc(ones1x128, (1, 128), 'o128', F32, sc_)
            ocol = lc(onescol, (128, 1), 'ocol', BF16, sc_)
            idn = lc(ident, (128, 128), 'idn', F32, sc_)
            idnb = lc(identb, (128, 128), 'idnb', BF16, sc_)
            w1S = lc(w1C, (CQ, NQ * H), 'w1S', BF16, sc_)
            w2S = lc(w2C, (128, HT * DC), 'w2S', BF16, sc_)
            WkS = lc(WkC, (128, RT * DC), 'WkS', BF16, sc_)
            WvS = lc(WvC, (128, RT * DC), 'WvS', BF16, sc_)
            epsT = pc.tile([128, 1], F32, name='epsT')
            nc.gpsimd.memset(epsT[:], EPS)

            # =========== PHASE 1 ===========
            # conv: 50 chunks of 512 cols; output rows 16 = (g, kv, c)
            # staged in 5 groups of 10 chunks to bound SBUF usage
            convT = {0: pb.tile([128, RT * T], BF16, name='convT0'),
                     1: pb.tile([128, RT * T], BF16, name='convT1')}
            sqw = {0: pb.tile([128, RT * T], BF16, name='sqw0'),
                   1: pb.tile([128, RT * T], BF16, name='sqw1')}
            invP = {0: pb.tile([128, RT * T], F32, name='invP0'),
                    1: pb.tile([128, RT * T], F32, name='invP1')}
            nkT = {0: pb.tile([128, RT * T], BF16, name='nkT0'),
                   1: pb.tile([128, RT * T], BF16, name='nkT1')}
            RG = 5                      # r-tiles per slab (== CPAIR/2)
            for gi in range(RT // RG):
                gsl = slice(gi * RG * T, (gi + 1) * RG * T)
                cg = px.tile([16, 16 * RG * T], BF16, name='cg', tag='cg',
                             bufs=2)
                cg4 = cg[:].rearrange('p (j r t) -> p j r t', j=16, r=RG)
                xt = px.tile([XROWS, CPAIR * 512], BF16, name='xch',
                             tag='xch', bufs=2)
                nc.sync.dma_start(xt[:],
                                  X80[gi * XROWS:(gi + 1) * XROWS, :])
                for ni in range(2 * RG):
                    half, rl = ni % 2, ni // 2
                    ps = psA.tile([16, 512], F32, name='cps', tag='cps')
                    nc.tensor.matmul(
                        ps[:], W73s[:],
                        xt[0:73, ni * 512:(ni + 1) * 512],
                        start=True, stop=True)
                    ps3 = ps[:].rearrange('p (j t) -> p j t', j=8)
                    dst3 = cg4[:, half * 8:(half + 1) * 8, rl, :]
                    if half == 0:
                        nc.scalar.activation(dst3, ps3, AF.Copy)
                    else:
                        nc.vector.tensor_copy(dst3, ps3)
                # scatter group -> convT r-window, then rms for the window
                for kv in range(2):
                    for g in range(2):
                        for c in range(4):
                            row = g * 8 + kv * 4 + c
                            dst = convT[kv][:].rearrange(
                                '(g c j) (r t) -> g c j r t', g=2, c=4, r=RT)
                            nc.gpsimd.dma_start(
                                dst[g, c, :, gi * RG:(gi + 1) * RG, :],
                                cg[row:row + 1, :])
                for kv in range(2):
                    nc.gpsimd.tensor_tensor(sqw[kv][:, gsl],
                                            convT[kv][:, gsl],
                                            convT[kv][:, gsl], OP.mult)
                    ss2 = psB.tile([128, RG * T], F32, name='ss2', tag='acc')
                    for rl in range(RG):
                        r = gi * RG + rl
                        nc.tensor.matmul(ss2[:, rl * T:(rl + 1) * T], S4s[:],
                                         sqw[kv][:, r * T:(r + 1) * T],
                                         start=True, stop=True)
                    nc.scalar.activation(invP[kv][:, gsl], ss2[:],
                                         AF.Sqrt, bias=epsT[:], scale=0.25)
                    nc.vector.reciprocal(invP[kv][:, gsl], invP[kv][:, gsl])
                    rp = rpk if kv == 0 else rpv
                    nc.vector.scalar_tensor_tensor(
                        nkT[kv][:, gsl], convT[kv][:, gsl], rp[:],
                        invP[kv][:, gsl], OP.mult, OP.mult)
            if debug:
                nc.sync.dma_start(d_nkT[:], nkT[0][:])

            # dense: keys/vals [T, DC] (T on partitions), weights preloaded
            kv_sb = {}
            for kv, (Wsb, bS) in enumerate(((WkS, bkS), (WvS, bvS))):
                ps = psA.tile([T, DC], F32, name='dps', tag='dps', bufs=1)
                for r in range(RT):
                    nc.tensor.matmul(ps[:],
                                     nkT[kv][:, r * T:(r + 1) * T],
                                     Wsb[:, r * DC:(r + 1) * DC],
                                     start=(r == 0), stop=False)
                nc.tensor.matmul(ps[:], o64[:],
                                 bS[:], start=False, stop=True)
                sb = pk.tile([T, DC], BF16, name=f'kv{kv}')
                nc.vector.tensor_copy(sb[:], ps[:])
                kv_sb[kv] = sb
            keys, vals = kv_sb[0], kv_sb[1]
            if debug:
                nc.sync.dma_start(d_keys[:], keys[:])
                nc.sync.dma_start(d_vals[:], vals[:])

            # transpose keys -> keysT chunks [98, 64] x4
            keysT = pk.tile([CQ, NQ * T], BF16, name='keysT')
            for q in range(NQ):
                pt = psT.tile([CQ, T], BF16, name='tps', tag='mmT')
                nc.tensor.transpose(pt[:], keys[:, q * CQ:(q + 1) * CQ],
                                    idnb[0:T, 0:T])
                nc.vector.tensor_copy(keysT[:, q * T:(q + 1) * T], pt[:])

            # scb = bcast(sc), scb2, q2 = vals*scb, scv = scb*vals transposed
            psc = psA.tile([T, DC], F32, name='pscb', tag='dps', bufs=1)
            nc.tensor.matmul(psc[:], o64[:], scS[:],
                             start=True, stop=True)
            scb = pk.tile([T, DC], F32, name='scb')
            nc.vector.tensor_copy(scb[:], psc[:])
            scb2 = pk.tile([T, DC], F32, name='scb2')
            nc.vector.tensor_tensor(scb2[:], scb[:], scb[:], OP.mult)
            q2 = pk.tile([T, DC], BF16, name='q2')
            nc.vector.tensor_tensor(q2[:], vals[:], scb[:], OP.mult)
            P3 = pk.tile([CQ, NQ * 3 * T], BF16, name='P3')
            for q in range(NQ):
                pt = psT.tile([CQ, T], BF16, name='tps', tag='mmT')
                nc.tensor.transpose(pt[:], q2[:, q * CQ:(q + 1) * CQ],
                                    idnb[0:T, 0:T])
                nc.vector.tensor_copy(
                    P3[:, (q * 3 + 1) * T:(q * 3 + 2) * T], pt[:])

            # w2T chunks [98, 512] x4 (PE transposes)
            w2T = pk.tile([CQ, NQ * H], BF16, name='w2T')
            for q in range(NQ):
                for m in range(HT):
                    pt = psT.tile([CQ, 128], BF16, name='t2ps', tag='mmT')
                    nc.tensor.transpose(
                        pt[:], w2S[:, m * DC + q * CQ:
                                   m * DC + (q + 1) * CQ], idnb[:])
                    nc.vector.tensor_copy(
                        w2T[:, q * H + m * 128:q * H + (m + 1) * 128], pt[:])

            # G_K = keys keys^T  (accumulate over chunks)
            pgk = psB.tile([T, T], F32, name='pgk', tag='acc')
            for q in range(NQ):
                nc.tensor.matmul(pgk[:], keysT[:, q * T:(q + 1) * T],
                                 keysT[:, q * T:(q + 1) * T],
                                 start=(q == 0), stop=(q == NQ - 1))
            GK = pk.tile([T, T], BF16, name='GK')
            nc.vector.tensor_copy(GK[:], pgk[:])
            if debug:
                nc.sync.dma_start(d_GK[:], GK[:])

            # z1T partial [H(4x128), T] = w1C^T keysT + b1/8 (one PSUM bank)
            z1ps = psB.tile([128, HT * T], F32, name='z1ps', tag='acc')
            for m in range(HT):
                msl = slice(m * T, (m + 1) * T)
                for q in range(NQ):
                    nc.tensor.matmul(z1ps[:, msl],
                                     w1S[:, q * H + m * 128:
                                         q * H + (m + 1) * 128],
                                     keysT[:, q * T:(q + 1) * T],
                                     start=(q == 0), stop=False)
                nc.tensor.matmul(z1ps[:, msl], b1r8[:, m * 128:(m + 1) * 128],
                                 o64[:], start=False, stop=True)
            z1Tp = pk.tile([128, HT * T], BF16, name='z1Tp')
            nc.vector.tensor_copy(z1Tp[:], z1ps[:])

            # ---- R1: AllReduce [128, 256 z1T cols | 32 GK cols] bf16 ----
            r1i = pd.tile([128, HT * T + 32], BF16, name='r1i')
            r1o = pd.tile([128, HT * T + 32], BF16, name='r1o')
            nc.gpsimd.dma_start(r1i[:, 0:HT * T], z1Tp[:])
            nc.gpsimd.dma_start(
                r1i[:, HT * T:HT * T + 32].rearrange('(p h) c -> p h c',
                                                     h=2),
                GK[:].rearrange('p (h c) -> p h c', h=2))
            nc.gpsimd.collective_compute(
                'AllReduce', OP.add, replica_groups=[list(range(NCORES))],
                ins=[r1i.opt()], outs=[r1o.opt()])

            z1T = pk.tile([128, HT * T], BF16, name='z1T')
            nc.sync.dma_start(z1T[:], r1o[:, 0:HT * T])
            GKg = pk.tile([T, T], BF16, name='GKg')
            nc.sync.dma_start(
                GKg[:].rearrange('p (h c) -> p h c', h=2),
                r1o[:, HT * T:HT * T + 32].rearrange('(p h) c -> p h c',
                                                     h=2))
            if debug:
                for m in range(HT):
                    nc.sync.dma_start(d_z1T[m * 128:(m + 1) * 128, :],
                                      z1T[:, m * T:(m + 1) * T])

            # R64 = diag(wv) @ (GK + 1)  (for z1f correction incl. agg_b1)
            R64 = pk.tile([T, T], BF16, name='R64')
            nc.vector.tensor_scalar(R64[:], GKg[:], 1.0, wvS[:],
                                    OP.add, OP.mult)

            # =========== PHASE 2 ===========
            hT = pk.tile([128, HT * T], BF16, name='hT')
            nc.scalar.activation(hT[:], z1T[:], AF.Gelu_apprx_tanh)
            # h [T, H]
            h = pk.tile([T, H], BF16, name='h')
            for m in range(HT):
                pt = psT.tile([T, 128], BF16, name='hps', tag='mmT')
                nc.tensor.transpose(pt[:], hT[:, m * T:(m + 1) * T], idnb[:])
                nc.vector.tensor_copy(h[:, m * 128:(m + 1) * 128], pt[:])

            # y = h @ w2C  [T, DC]
            py = psA.tile([T, DC], F32, name='py', tag='dps', bufs=1)
            for m in range(HT):
                nc.tensor.matmul(py[:], hT[:, m * T:(m + 1) * T],
                                 w2S[:, m * DC:(m + 1) * DC],
                                 start=(m == 0), stop=(m == HT - 1))
            y = pk.tile([T, DC], BF16, name='y')
            nc.vector.tensor_copy(y[:], py[:])
            if debug:
                nc.sync.dma_start(d_y[:], y[:])

            # yT chunks + (sc^2 y)T into P3 slots (i=2: yT, i=0: s2yT)
            for q in range(NQ):
                ysl = slice((q * 3 + 2) * T, (q * 3 + 3) * T)
                pt = psT.tile([CQ, T], BF16, name='tps', tag='mmT')
                nc.tensor.transpose(pt[:], y[:, q * CQ:(q + 1) * CQ],
                                    idnb[0:T, 0:T])
                nc.vector.tensor_copy(P3[:, ysl], pt[:])
                nc.vector.tensor_scalar(P3[:, (q * 3) * T:(q * 3 + 1) * T],
                                        P3[:, ysl],
                                        scsqTS[:, q:q + 1], None,
                                        OP.mult)

            # scalars C = sum y^2, A = sum (scb y)^2, B = sum (scb y) v
            ua = pk.tile([T, DC], F32, name='ua')
            nc.vector.tensor_tensor(ua[:], y[:], scb[:], OP.mult)
            scr = pk.tile([T, DC], F32, name='scr')
            Cc = pk.tile([T, 1], F32, name='Cc')
            Ac = pk.tile([T, 1], F32, name='Ac')
            Bc = pk.tile([T, 1], F32, name='Bc')
            nc.scalar.activation(scr[:], y[:], AF.Square, accum_out=Cc[:])
            nc.scalar.activation(scr[:], ua[:], AF.Square, accum_out=Ac[:])
            nc.vector.scalar_tensor_tensor(scr[:], ua[:], 1.0, vals[:],
                                           OP.mult, OP.mult,
                                           accum_out=Bc[:])

            # P matmuls: out block (m) = [P1m | P2m | P3m], shared stationary
            Pt = pk.tile([128, 3 * HT * T], BF16, name='Pt')
            for m in range(HT):
                pp = psB.tile([128, 3 * T], F32, name='pp', tag='acc')
                for q in range(NQ):
                    nc.tensor.matmul(
                        pp[:],
                        w2T[:, q * H + m * 128:q * H + (m + 1) * 128],
                        P3[:, q * 3 * T:(q + 1) * 3 * T],
                        start=(q == 0), stop=(q == NQ - 1))
                nc.vector.tensor_copy(
                    Pt[:, m * 3 * T:(m + 1) * 3 * T], pp[:])

            # ---- R2: AllReduce [128, 768 P cols | 3 C/A/B cols] bf16 ----
            NP2 = 3 * HT * T
            r2i = pd.tile([128, NP2 + 3], BF16, name='r2i')
            r2o = pd.tile([128, NP2 + 3], BF16, name='r2o')
            nc.gpsimd.dma_start(r2i[:, 0:NP2], Pt[:])
            nc.gpsimd.dma_start(r2i[0:T, NP2 + 0:NP2 + 1], Cc[:])
            nc.gpsimd.dma_start(r2i[0:T, NP2 + 1:NP2 + 2], Ac[:])
            nc.gpsimd.dma_start(r2i[0:T, NP2 + 2:NP2 + 3], Bc[:])
            nc.gpsimd.collective_compute(
                'AllReduce', OP.add, replica_groups=[list(range(NCORES))],
                ins=[r2i.opt()], outs=[r2o.opt()])

            Pg = pk.tile([128, 3 * HT * T], BF16, name='Pg')
            nc.sync.dma_start(Pg[:], r2o[:, 0:NP2])
            CAB = pk.tile([T, 3], BF16, name='CAB')
            nc.sync.dma_start(CAB[:], r2o[0:T, NP2:NP2 + 3])

            # scalar chain, column space [T, 1] (cols of scol: inv a1 a2 a3)
            scol = pk.tile([T, 4], F32, name='scol')
            i2c = pk.tile([T, 1], F32, name='i2c')
            t1c = pk.tile([T, 1], F32, name='t1c')
            Scc = pk.tile([T, 1], F32, name='Scc')
            nc.scalar.activation(scol[:, 0:1], CAB[:, 0:1], AF.Sqrt,
                                 bias=epsT[0:T, :], scale=1.0 / D)
            nc.vector.reciprocal(scol[:, 0:1], scol[:, 0:1])
            nc.vector.tensor_tensor(i2c[:], scol[:, 0:1], scol[:, 0:1],
                                    OP.mult)
            # S = 2 inv A - 2 B
            nc.vector.scalar_tensor_tensor(t1c[:], scol[:, 0:1], 2.0,
                                           CAB[:, 1:2], OP.mult, OP.mult)
            nc.vector.scalar_tensor_tensor(Scc[:], CAB[:, 2:3], -2.0,
                                           t1c[:], OP.mult, OP.add)
            # a1 = 2 inv^2 ; a2 = 2 inv ; a3 = inv^3 S / D
            nc.vector.tensor_scalar(scol[:, 1:2], i2c[:], 2.0, None, OP.mult)
            nc.vector.tensor_scalar(scol[:, 2:3], scol[:, 0:1], 2.0, None,
                                    OP.mult)
            nc.vector.scalar_tensor_tensor(t1c[:], i2c[:], 1.0 / D,
                                           scol[:, 0:1], OP.mult, OP.mult)
            nc.vector.tensor_tensor(scol[:, 3:4], t1c[:], Scc[:], OP.mult)

            # a1/a2/a3 to a single [1, 3T] row, then broadcast ab2 [128, 3T]
            r3ps = psB.tile([1, 3 * T], F32, name='r3ps', tag='mm64', bufs=1)
            for j in range(3):
                nc.tensor.matmul(r3ps[:, j * T:(j + 1) * T],
                                 scol[:, 1 + j:2 + j], idn[0:T, 0:T],
                                 start=True, stop=True)
            arow = pk.tile([1, 3 * T], F32, name='arow')
            nc.vector.tensor_copy(arow[:], r3ps[:])
            abps = psB.tile([128, 3 * T], F32, name='abps', tag='mm64',
                            bufs=1)
            for j in range(3):
                nc.tensor.matmul(abps[:, j * T:(j + 1) * T], o128[:],
                                 arow[:, j * T:(j + 1) * T],
                                 start=True, stop=True)
            ab2 = pk.tile([128, 3 * T], F32, name='ab2')
            nc.vector.tensor_copy(ab2[:], abps[:])

            # dhT = a1*P1 - a2*P2 - a3*P3 ; dz1T = dhT * gelu'(z1T)
            dgel = pk.tile([128, HT * T], BF16, name='dgel')
            nc.scalar.activation(dgel[:], z1T[:], AF.Derivative_Gelu)
            dhT = pk.tile([128, HT * T], F32, name='dhT')
            tmpA = pk.tile([128, 3 * T], F32, name='tmpA')
            for m in range(HT):
                msl = slice(m * T, (m + 1) * T)
                nc.vector.tensor_tensor(tmpA[:],
                                        Pg[:, m * 3 * T:(m + 1) * 3 * T],
                                        ab2[:], OP.mult)
                nc.vector.tensor_tensor(dhT[:, msl], tmpA[:, 0:T],
                                        tmpA[:, T:2 * T], OP.subtract)
                nc.vector.tensor_tensor(dhT[:, msl], dhT[:, msl],
                                        tmpA[:, 2 * T:3 * T], OP.subtract)
            dz1T = pk.tile([128, HT * T], BF16, name='dz1T')
            nc.vector.tensor_tensor(dz1T[:], dhT[:], dgel[:], OP.mult)

            # dz1 [T, H]
            dz1 = pk.tile([T, H], BF16, name='dz1')
            for m in range(HT):
                pt = psT.tile([T, 128], BF16, name='dzps', tag='mmT')
                nc.tensor.transpose(pt[:], dz1T[:, m * T:(m + 1) * T], idnb[:])
                nc.vector.tensor_copy(dz1[:, m * 128:(m + 1) * 128], pt[:])

            # z1fT = z1T - dz1^T-weighted: T2T[m] = dz1[:,m]^T @ R64
            t2ps = psB.tile([128, HT * T], F32, name='t2t', tag='mm64',
                            bufs=1)
            for m in range(HT):
                nc.tensor.matmul(t2ps[:, m * T:(m + 1) * T],
                                 dz1[:, m * 128:(m + 1) * 128],
                                 R64[:], start=True, stop=True)
            z1fT = pk.tile([128, HT * T], F32, name='z1fT')
            nc.vector.tensor_tensor(z1fT[:], z1T[:], t2ps[:], OP.subtract)
            hfT = pk.tile([128, HT * T], BF16, name='hfT')
            nc.scalar.activation(hfT[:], z1fT[:], AF.Gelu_apprx_tanh)

            # G = a1*(scb2*y) - a2*(q2) - a3*y  (column scalars)
            G = pk.tile([T, DC], F32, name='G')
            gt1 = pk.tile([T, DC], F32, name='gt1')
            nc.vector.tensor_tensor(gt1[:], y[:], scb2[:], OP.mult)
            nc.vector.tensor_scalar(G[:], gt1[:], scol[:, 1:2], None, OP.mult)
            nc.vector.tensor_scalar(gt1[:], q2[:], scol[:, 2:3], None, OP.mult)
            nc.vector.tensor_tensor(G[:], G[:], gt1[:], OP.subtract)
            nc.vector.tensor_scalar(gt1[:], y[:], scol[:, 3:4], None, OP.mult)
            nc.vector.tensor_tensor(G[:], G[:], gt1[:], OP.subtract)

            # agg_w2 & w2' = w2 - h^T (wv*G)
            wG = pk.tile([T, DC], BF16, name='wG')
            nc.vector.tensor_scalar(wG[:], G[:], wvS[:], None, OP.mult)
            w2p = pk.tile([128, HT * DC], BF16, name='w2p')
            for m in range(HT):
                pa = psA.tile([128, DC], F32, name='paw2', tag='dps', bufs=1)
                nc.tensor.matmul(pa[:],
                                 h[:, m * 128:(m + 1) * 128],
                                 wG[:], start=True, stop=True)
                nc.vector.tensor_tensor(w2p[:, m * DC:(m + 1) * DC],
                                        w2S[:, m * DC:(m + 1) * DC], pa[:],
                                        OP.subtract)
                if debug:
                    nc.sync.dma_start(d_w2p[m * 128:(m + 1) * 128, :],
                                      w2p[:, m * DC:(m + 1) * DC])

            # rows: b2' ; sc' ; sc'*ros (all partition-0 tiles)
            brow = pk.tile([1, 3 * DC], F32, name='brow')
            pr = psB.tile([1, DC], F32, name='prow', tag='acc')
            nc.tensor.matmul(pr[:], wvS[:], G[:],
                             start=True, stop=True)
            nc.vector.tensor_tensor(brow[:, 0:DC], b2S[:], pr[:], OP.subtract)

            # r2y = 2*inv*(scb*y)*y - 2*v*y ; agg_sc = (wv*inv)^T r2y
            nc.vector.tensor_tensor(gt1[:], ua[:], y[:], OP.mult)
            nc.vector.tensor_scalar(gt1[:], gt1[:], scol[:, 2:3], None, OP.mult)
            r2y2 = pk.tile([T, DC], F32, name='r2y2')
            nc.vector.tensor_tensor(r2y2[:], vals[:], y[:], OP.mult)
            nc.vector.tensor_scalar(r2y2[:], r2y2[:], 2.0, None, OP.mult)
            nc.vector.tensor_tensor(gt1[:], gt1[:], r2y2[:], OP.subtract)
            wiv = pk.tile([T, 1], F32, name='wiv')
            nc.vector.tensor_tensor(wiv[:], wvS[:], scol[:, 0:1], OP.mult)
            pr2 = psB.tile([1, DC], F32, name='prow2', tag='acc')
            nc.tensor.matmul(pr2[:], wiv[:],
                             gt1[:], start=True, stop=True)
            nc.vector.tensor_tensor(brow[:, DC:2 * DC], scS32[:], pr2[:],
                                    OP.subtract)
            nc.vector.tensor_tensor(brow[:, 2 * DC:3 * DC],
                                    brow[:, DC:2 * DC], rosS[:], OP.mult)

            # transpose rows to columns: colrows[:, q*3+j]
            colrows = pk.tile([CQ, NQ * 3], F32, name='colrows')
            for q in range(NQ):
                pt = psB.tile([CQ, 3], F32, name='crps', tag='mm64', bufs=1)
                for j in range(3):
                    nc.tensor.transpose(
                        pt[:, j:j + 1],
                        brow[:, j * DC + q * CQ:j * DC + (q + 1) * CQ],
                        idn[0:1, 0:1])
                nc.vector.tensor_copy(colrows[:, q * 3:(q + 1) * 3], pt[:])

            # yfT chunks [98, T] = w2p^T @ hfT + b2'T ; squares and partials
            yfT = pk.tile([CQ, NQ * T], F32, name='yfT')
            sqf = pk.tile([CQ, NQ * T], BF16, name='sqf')
            ssqf = pk.tile([CQ, NQ * T], BF16, name='ssqf')
            for q in range(NQ):
                pf = psB.tile([CQ, T], F32, name='pyf', tag='acc')
                for m in range(HT):
                    nc.tensor.matmul(pf[:],
                                     w2p[:, m * DC + q * CQ:m * DC + (q + 1) * CQ],
                                     hfT[:, m * T:(m + 1) * T],
                                     start=(m == 0), stop=(m == HT - 1))
                sl = slice(q * T, (q + 1) * T)
                nc.vector.tensor_scalar(yfT[:, sl], pf[:],
                                        colrows[:, q * 3:q * 3 + 1], None,
                                        OP.add)
                nc.vector.tensor_tensor(sqf[:, sl], yfT[:, sl], yfT[:, sl],
                                        OP.mult)
                nc.vector.tensor_scalar(ssqf[:, sl], yfT[:, sl],
                                        colrows[:, q * 3 + 1:q * 3 + 2], None,
                                        OP.mult)
                nc.vector.tensor_tensor(ssqf[:, sl], ssqf[:, sl], ssqf[:, sl],
                                        OP.mult)
            if debug:
                for q in range(NQ):
                    nc.sync.dma_start(d_yfT[q * CQ:(q + 1) * CQ, :],
                                      yfT[:, q * T:(q + 1) * T])
            pfin = psB.tile([1, 2 * T], F32, name='pfin', tag='acc')
            for q in range(NQ):
                nc.tensor.matmul(pfin[:, 0:T], ocol[0:CQ, :],
                                 sqf[:, q * T:(q + 1) * T],
                                 start=(q == 0), stop=(q == NQ - 1))
            for q in range(NQ):
                nc.tensor.matmul(pfin[:, T:2 * T], ocol[0:CQ, :],
                                 ssqf[:, q * T:(q + 1) * T],
                                 start=(q == 0), stop=(q == NQ - 1))
            fin = pk.tile([1, 2 * T], F32, name='fin')
            nc.vector.tensor_copy(fin[:], pfin[:])

            # ---- R3: AllReduce final scalars ----
            r3i = pd.tile([1, 2 * T], F32, name='r3i')
            r3o = pd.tile([1, 2 * T], F32, name='r3o')
            nc.gpsimd.dma_start(r3i[:], fin[:])
            nc.gpsimd.collective_compute(
                'AllReduce', OP.add, replica_groups=[list(range(NCORES))],
                ins=[r3i.opt()], outs=[r3o.opt()])

            # invf = rsqrt(Cf/D + eps); invp = rsqrt(invf^2 * Af/D + eps)
            CfAf = pk.tile([1, 2 * T], F32, name='CfAf')
            nc.sync.dma_start(CfAf[:], r3o[:])
            invft = pk.tile([1, T], F32, name='invft')
            invpt = pk.tile([1, T], F32, name='invpt')
            fft = pk.tile([1, T], F32, name='fft')
            nc.scalar.activation(invft[:], CfAf[:, 0:T], AF.Sqrt,
                                 bias=epsT[0:1, :], scale=1.0 / D)
            nc.vector.reciprocal(invft[:], invft[:])
            nc.vector.tensor_tensor(invpt[:], invft[:], invft[:], OP.mult)
            nc.vector.tensor_tensor(invpt[:], invpt[:], CfAf[:, T:2 * T],
                                    OP.mult)
            nc.scalar.activation(invpt[:], invpt[:], AF.Sqrt,
                                 bias=epsT[0:1, :], scale=1.0 / D)
            nc.vector.reciprocal(invpt[:], invpt[:])
            nc.vector.tensor_tensor(fft[:], invft[:], invpt[:], OP.mult)
            ffb = pk.tile([128, T], F32, name='ffb')
            pt = psB.tile([128, T], F32, name='ffps', tag='mm64', bufs=1)
            nc.tensor.matmul(pt[:], o128[:], fft[:], start=True, stop=True)
            nc.vector.tensor_copy(ffb[:], pt[:])

            # out = yfT * scrosT * ff  (single [CQ, NQ*T] store)
            outsb = pk.tile([CQ, NQ * T], F32, name='outsb')
            for q in range(NQ):
                sl = slice(q * T, (q + 1) * T)
                nc.vector.scalar_tensor_tensor(
                    outsb[:, sl], yfT[:, sl],
                    colrows[:, q * 3 + 2:q * 3 + 3], ffb[0:CQ, :],
                    OP.mult, OP.mult)
            nc.sync.dma_start(out[:], outsb[:])

    _split_excess_waits(nc)
    return nc, sorted(dbg_outs.keys())


def make_inputs(inputs):
    """Build the 8 per-core input dicts from the full problem inputs."""
    x = np.asarray(inputs['x'], np.float32)
    x_t = np.transpose(x, (0, 2, 3, 1))
    X72 = _build_im2col(x_t)
    W73 = _build_w73(np.asarray(inputs['conv_k_w'], np.float32),
                     np.asarray(inputs['conv_k_b'], np.float32),
                     np.asarray(inputs['conv_v_w'], np.float32),
                     np.asarray(inputs['conv_v_b'], np.float32))
    perm, valid = _din_perm()
    dkw = np.asarray(inputs['dense_k_w'], np.float32)
    dvw = np.asarray(inputs['dense_v_w'], np.float32)
    Wk_full = np.zeros((DINP, D), np.float32)
    Wv_full = np.zeros((DINP, D), np.float32)
    Wk_full[valid] = dkw[perm[valid]]
    Wv_full[valid] = dvw[perm[valid]]

    w1 = np.asarray(inputs['mem_w1'], np.float32)
    w2 = np.asarray(inputs['mem_w2'], np.float32)
    sc = np.asarray(inputs['mem_scale'], np.float32)
    ros = np.asarray(inputs['rms_out_scale'], np.float32)
    dkb = np.asarray(inputs['dense_k_b'], np.float32)
    dvb = np.asarray(inputs['dense_v_b'], np.float32)
    b1 = np.asarray(inputs['mem_b1'], np.float32)
    b2 = np.asarray(inputs['mem_b2'], np.float32)

    base = {
        'X80': _pack_slabs(X72).astype(NPBF), 'W73': W73.astype(NPBF),
        'b1row8': (b1 / NCORES).reshape(1, H).astype(NPBF),
        'rmspk': _rms_pattern(np.asarray(inputs['rms_k_scale'], np.float32)),
        'rmspv': _rms_pattern(np.asarray(inputs['rms_v_scale'], np.float32)),
        'S4': _s4().astype(NPBF), 'wv': _wvec().reshape(T, 1),
        'ones1x64': np.ones((1, T), NPBF),
        'ones1x128': np.ones((1, 128), np.float32),
        'onescol': np.ones((128, 1), NPBF),
        'ident': np.eye(128, dtype=np.float32),
        'identb': np.eye(128, dtype=np.float32).astype(NPBF),
    }
    in_maps = []
    for c in range(NCORES):
        sl = slice(c * DC, (c + 1) * DC)
        m = dict(base)
        m['WkC'] = np.ascontiguousarray(
            Wk_full[:, sl].reshape(RT, 128, DC).transpose(1, 0, 2)
            .reshape(128, RT * DC)).astype(NPBF)
        m['WvC'] = np.ascontiguousarray(
            Wv_full[:, sl].reshape(RT, 128, DC).transpose(1, 0, 2)
            .reshape(128, RT * DC)).astype(NPBF)
        m['bkC'] = dkb[sl].reshape(1, DC).astype(NPBF)
        m['bvC'] = dvb[sl].reshape(1, DC).astype(NPBF)
        w1c = w1[sl, :]
        m['w1C'] = np.ascontiguousarray(
            w1c.reshape(NQ, CQ, H).transpose(1, 0, 2)
            .reshape(CQ, NQ * H)).astype(NPBF)
        w2c = w2[:, sl]
        m['w2C'] = np.ascontiguousarray(
            w2c.reshape(HT, 128, DC).transpose(1, 0, 2)
            .reshape(128, HT * DC)).astype(NPBF)
        m['b2C'] = b2[sl].reshape(1, DC)
        m['scC'] = sc[sl].reshape(1, DC).astype(NPBF)
        m['scC32'] = sc[sl].reshape(1, DC)
        m['rosC'] = ros[sl].reshape(1, DC)
        m['scsqT'] = np.ascontiguousarray(
            (sc[sl] ** 2).reshape(NQ, CQ).T)
        in_maps.append(m)
    return in_maps


def kernel(**inputs):
    if 'nc' not in _NC_CACHE:
        _NC_CACHE['nc'], _ = build_nc(debug=False)
    nc = _NC_CACHE['nc']
    in_maps = make_inputs(inputs)
    res = run_bass_kernel_spmd(nc, in_maps, list(range(NCORES)))
    blocks = [res.results[c]['out'].reshape(CQ, NQ, T).transpose(1, 0, 2)
              .reshape(DC, T) for c in range(NCORES)]
    YT = np.concatenate(blocks, axis=0)
    return np.ascontiguousarray(YT.T).reshape(T, 4, 28, 28)



# revision 46
# speedup vs baseline: 1.0309x; 1.0309x over previous
"""Trainium2 Bass kernel for nn_MirasModel (scatter_memory).

Strategy (8 NeuronCores, SPMD):
  - Column-shard the shared D=3136 feature dimension: core c owns Dc=392
    columns of dense_k_w / dense_v_w / mem_w2 / biases / scales, and the
    matching 392 rows of mem_w1.
  - Conv + rmsnorm computed fully on every core (tiny) via a packed
    im2col matmul, with a DMA scatter producing the transposed
    [Din, T] activation layout the dense matmuls need.
  - Three AllReduce rounds:
      R1: z1 = keys@w1+b1 partial sums  +  Gram(keys) = keys keys^T
      R2: per-token scalars (C,A,B) + backward projections P1,P2,P3
      R3: final-forward rmsnorm scalar partials
    The Gram matrix lets z1f = z1 - G_K diag(w) dz1 be computed locally,
    eliminating a fourth round (keys @ agg_w1 == Gram @ diag(w) @ dz1).
  - All heavy DMA (im2col + dense weight shards) hides under R1's
    collective entry latency.
"""

import sys

if '/opt/trn_rl_repo' not in sys.path:
    sys.path.insert(0, '/opt/trn_rl_repo')

import numpy as np

import concourse.bass as bass
import concourse.mybir as mybir
from concourse import tile
from concourse.bass_utils import run_bass_kernel_spmd

F32 = mybir.dt.float32
F32R = mybir.dt.float32r
BF16 = mybir.dt.bfloat16
NPBF = mybir.dt.np(mybir.dt.bfloat16)
AF = mybir.ActivationFunctionType
OP = mybir.AluOpType

T = 64
D = 3136
H = 512
NCORES = 8
DC = D // NCORES            # 392 columns per core
CQ = 98                     # Dc sub-chunk (4 per core)
NQ = DC // CQ               # 4
PPIX = 800                  # padded pixel count (784 real + 16 dummy)
DINP = PPIX * 4             # padded Din = 3200
RT = DINP // 128            # 25 Din tiles
NPTR = PPIX // 2            # 400 pixel-pairs
NCONV = NPTR * T // 512     # 50 conv matmul chunks
CPAIR = 10                  # conv chunks per DMA slab (one r-group)
NSLAB = NCONV // CPAIR      # 25 slabs
XROWS = 80                  # 73 im2col rows padded to 80 (16 | 80*512)
HT = H // 128               # 4 H tiles
ALPHA, ETA0, EPS = 0.9, 0.1, 1e-6

_NC_CACHE = {}


# ---------------------------------------------------------------------------
# walrus workaround: this compiler build rejects Drain instructions carrying
# more than one sync wait; split extras onto preceding Drains.
def _split_excess_waits(nc):
    """This walrus build has tight per-instruction sync-wait budgets
    (1 for Drain/Matmult/etc).  Move excess waits onto preceding NoOps."""
    LIM1 = 1

    def limit_for(ins):
        return LIM1

    n_new = 0
    for fn in nc.m.functions:
        for bb in fn.blocks:
            i = 0
            while i < len(bb.instructions):
                ins = bb.instructions[i]
                si = getattr(ins, 'sync_info', None)
                lim = limit_for(ins)
                if (si is not None and si.on_wait and len(si.on_wait) > lim
                        and getattr(ins, 'engine', None) is not None):
                    waits = list(si.on_wait)
                    keep, extra = waits[:lim], waits[lim:]
                    ins.sync_info = mybir.SyncInfo(on_wait=keep,
                                                  on_update=si.on_update)
                    pos = i
                    for j in range(0, len(extra), LIM1):
                        n_new += 1
                        nd = mybir.InstNoOp(
                            name=f"I-waitfix-{n_new}",
                            engine=ins.engine,
                            bass_nofuse=True,
                            sync_info=mybir.SyncInfo(
                                on_wait=extra[j:j + LIM1], on_update=[]),
                        )
                        bb.instructions.insert(pos, nd)
                        pos += 1
                        i += 1
                i += 1
    return n_new


def _din_perm():
    """Device Din row -> reference Din index (p*4+c), p,c of padded grid."""
    idx = np.zeros(DINP, np.int64)
    valid = np.zeros(DINP, bool)
    for r in range(RT):
        for i in range(128):
            g, c, jj = i // 64, (i % 64) // 16, i % 16
            p = 2 * (16 * r + jj) + g
            row = r * 128 + i
            if p < 784:
                idx[row] = p * 4 + c
                valid[row] = True
    return idx, valid


def _pack_slabs(X72):
    """Repack [73, NCONV*512] im2col into DMA-slab layout.

    Returns [NSLAB*XROWS, CPAIR*512]; slab s rows 0:73 = X72 cols
    s*CPAIR*512:(s+1)*CPAIR*512, rows 73:80 zero.  Row-contiguous slabs
    let the HWDGE spray descriptors across all 16 DMA engines (a
    strided source pins the whole transfer to one engine).
    """
    Xs = np.zeros((NSLAB * XROWS, CPAIR * 512), np.float32)
    v = X72.reshape(73, NSLAB, CPAIR * 512)
    for s in range(NSLAB):
        Xs[s * XROWS:s * XROWS + 73, :] = v[:, s, :]
    return Xs


def _build_im2col(x_t, pad_val=0.0):
    """x_t: (T, 28, 28, 4) NHWC.  Returns X72 [73, NPTR*64] fp32.

    row = g*36 + (di*3+dj)*4 + ci  (g in 0..1), row 72 = ones.
    col = ptr*64 + t, pixel p = 2*ptr + g (row-major 28x28, padded to 800).
    """
    xp = np.zeros((T, 30, 30, 4), np.float32)
    xp[:, 1:29, 1:29, :] = x_t
    X = np.zeros((73, NPTR * T), np.float32)
    p = np.arange(PPIX)
    pi, pj = p // 28, p % 28
    ok = p < 784
    for g in range(2):
        psel = p[(p % 2) == g]
        ptr = psel // 2
        pis, pjs, oks = pi[(p % 2) == g], pj[(p % 2) == g], ok[(p % 2) == g]
        for di in range(3):
            for dj in range(3):
                for ci in range(4):
                    row = g * 36 + (di * 3 + dj) * 4 + ci
                    vals = np.zeros((NPTR, T), np.float32)
                    vsel = xp[:, np.clip(pis + di, 0, 29),
                              np.clip(pjs + dj, 0, 29), ci]  # (T, NPTR)
                    vals[oks[: NPTR], :] = vsel.T[oks[: NPTR], :]
                    # dummy pixels (>=784) contribute garbage later discarded
                    X[row, :] = vals.reshape(-1)
    X[72, :] = 1.0
    return X


def _build_w73(conv_k_w, conv_k_b, conv_v_w, conv_v_b):
    """W73 [73, 16]; col = g*8 + kv*4 + co."""
    W = np.zeros((73, 16), np.float32)
    for g in range(2):
        for kv, (w, b) in enumerate(((conv_k_w, conv_k_b),
                                     (conv_v_w, conv_v_b))):
            for di in range(3):
                for dj in range(3):
                    for ci in range(4):
                        W[g * 36 + (di * 3 + dj) * 4 + ci,
                          g * 8 + kv * 4:g * 8 + kv * 4 + 4] = w[di, dj, ci, :]
            W[72, g * 8 + kv * 4:g * 8 + kv * 4 + 4] = b
    return W


def _rms_pattern(scale4):
    """[128,1] per-partition rms scale: partition i -> scale4[(i%64)//16]."""
    i = np.arange(128)
    return scale4[(i % 64) // 16].astype(np.float32).reshape(128, 1)


def _s4():
    """Dup-selector [128, 128]: S[i, o] = 1 iff (g, j) of i == (g, j) of o.

    Partition layout (g, c, j): g = i // 64, c = (i % 64) // 16, j = i % 16.
    The sumsq matmul with this stationary yields the per-pixel channel
    sum-of-squares already duplicated across the 4 c-slots."""
    i = np.arange(128)
    gj = (i // 64) * 16 + (i % 16)
    return (gj[:, None] == gj[None, :]).astype(np.float32)


def _wvec():
    betas = (np.float32(ALPHA) ** np.arange(T, dtype=np.float32)).astype(np.float32)
    etas = (np.float32(ETA0) * betas).astype(np.float32)
    weights = (etas * (betas[-1] / betas)).astype(np.float32)
    return (np.float32(1e-4) * weights).astype(np.float32)


def build_nc(debug=False):
    nc = bass.Bass()

    def inp(name, shape, dt=F32):
        return nc.dram_tensor(name, list(shape), dt, kind="ExternalInput")

    X80 = inp('X80', (NSLAB * XROWS, CPAIR * 512), BF16)
    # combo tensors (host-packed):
    #   CE (bf16 [128, 144]): S4 dup-selector | W73
    #   CF (f32 [128, 2]): rmspk | rmspv
    #   CA (bf16 [1, 1752]): bk | bv | sc | o64 | b1/8
    #   CB (f32 [1, 1304]): b2 | sc32 | ros | o128
    #   CC (bf16 [128, 23345]): idnb | ocol | w2 | w1(pad128) | Wk | Wv
    #   CD (f32 [128, 133]): idn | scsqT | wv
    CE = inp('CE', (128, 144), BF16)
    CF = inp('CF', (128, 2))
    CA = inp('CA', (1, 3 * DC + T + H), BF16)
    CB = inp('CB', (1, 3 * DC + 128))
    CD = inp('CD', (128, 133))
    CC = inp('CC', (128, 129 + HT * DC + NQ * H + 2 * RT * DC), BF16)

    out = nc.dram_tensor('out', [CQ, NQ * T], F32, kind="ExternalOutput")
    dbg_outs = {}

    def dbg(name, shape):
        if debug:
            dbg_outs[name] = nc.dram_tensor(name, list(shape), F32,
                                            kind="ExternalOutput")
        return dbg_outs.get(name)

    d_nkT = dbg('d_nkT', (128, RT * T))
    d_keys = dbg('d_keys', (T, DC))
    d_vals = dbg('d_vals', (T, DC))
    d_z1T = dbg('d_z1T', (H, T))
    d_GK = dbg('d_GK', (T, T))
    d_y = dbg('d_y', (T, DC))
    d_P = dbg('d_P', (3 * H, T))
    d_dhT = dbg('d_dhT', (H, T))
    d_z1fT = dbg('d_z1fT', (H, T))
    d_w2p = dbg('d_w2p', (H, DC))
    d_yfT = dbg('d_yfT', (DC, T))

    with tile.TileContext(nc) as tc:
        with (
            tc.tile_pool(name='consts', bufs=1) as pc,
            tc.tile_pool(name='wstream', bufs=4) as pw,
            tc.tile_pool(name='xstream', bufs=4) as px,
            tc.tile_pool(name='big', bufs=1) as pb,
            tc.tile_pool(name='work', bufs=1) as pk,
            tc.tile_pool(name='psA', bufs=2, space='PSUM') as psA,
            tc.tile_pool(name='psB', bufs=2, space='PSUM') as psB,
            tc.tile_pool(name='psT', bufs=2, space='PSUM') as psT,
            tc.tile_pool(name='dram', bufs=1, space='DRAM') as pd,
        ):
            # ---- constants to SBUF: 2 sync + 4 scalar-queue loads ----
            CEs = pc.tile([128, 144], BF16, name='CEs')
            nc.sync.dma_start(CEs[:], CE[:])
            CFs = pc.tile([128, 2], F32, name='CFs')
            nc.sync.dma_start(CFs[:], CF[:])
            CAs = pc.tile([1, 3 * DC + T + H], BF16, name='CAs')
            nc.scalar.dma_start(CAs[:], CA[:])
            CBs = pc.tile([1, 3 * DC + 128], F32, name='CBs')
            nc.scalar.dma_start(CBs[:], CB[:])
            CDs = pc.tile([128, 133], F32, name='CDs')
            nc.scalar.dma_start(CDs[:], CD[:])
            CCs = pc.tile([128, 129 + HT * DC + NQ * H + 2 * RT * DC], BF16,
                          name='CCs')
            nc.scalar.dma_start(CCs[:], CC[:])

            S4s = CEs[:, 0:128]
            W73s = CEs[0:73, 128:144]
            rpk = CFs[:, 0:1]
            rpv = CFs[:, 1:2]
            bkS = CAs[:, 0:DC]
            bvS = CAs[:, DC:2 * DC]
            scS = CAs[:, 2 * DC:3 * DC]
            o64 = CAs[:, 3 * DC:3 * DC + T]
            b1r8 = CAs[:, 3 * DC + T:3 * DC + T + H]
            b2S = CBs[:, 0:DC]
            scS32 = CBs[:, DC:2 * DC]
            rosS = CBs[:, 2 * DC:3 * DC]
            o128 = CBs[:, 3 * DC:3 * DC + 128]
            idn = CDs[:, 0:128]
            scsqTS = CDs[0:CQ, 128:132]
            wvS = CDs[0:T, 132:133]
            idnb = CCs[:, 0:128]
            ocol = CCs[:, 128:129]
            OW2 = 129
            OW1 = OW2 + HT * DC
            OWK = OW1 + NQ * H
            OWV = OWK + RT * DC
            w2S = CCs[:, OW2:OW2 + HT * DC]
            w1S = CCs[0:CQ, OW1:OW1 + NQ * H]
            WkS = CCs[:, OWK:OWK + RT * DC]
            WvS = CCs[:, OWV:OWV + RT * DC]
            epsT = pc.tile([128, 1], F32, name='epsT')
            nc.gpsimd.memset(epsT[:], EPS)

            # =========== PHASE 1 ===========
            # conv: 50 chunks of 512 cols; output rows 16 = (g, kv, c)
            # staged in 5 groups of 10 chunks to bound SBUF usage
            convT = {0: pb.tile([128, RT * T], BF16, name='convT0'),
                     1: pb.tile([128, RT * T], BF16, name='convT1')}
            sqw = {0: pb.tile([128, RT * T], BF16, name='sqw0'),
                   1: pb.tile([128, RT * T], BF16, name='sqw1')}
            invP = {0: pb.tile([128, RT * T], F32, name='invP0'),
                    1: pb.tile([128, RT * T], F32, name='invP1')}
            nkT = {0: pb.tile([128, RT * T], BF16, name='nkT0'),
                   1: pb.tile([128, RT * T], BF16, name='nkT1')}
            RG = 5                      # r-tiles per slab (== CPAIR/2)
            for gi in range(RT // RG):
                gsl = slice(gi * RG * T, (gi + 1) * RG * T)
                cg = px.tile([16, 16 * RG * T], BF16, name='cg', tag='cg',
                             bufs=2)
                cg4 = cg[:].rearrange('p (j r t) -> p j r t', j=16, r=RG)
                xt = px.tile([XROWS, CPAIR * 512], BF16, name='xch',
                             tag='xch', bufs=2)
                nc.sync.dma_start(xt[:],
                                  X80[gi * XROWS:(gi + 1) * XROWS, :])
                for ni in range(2 * RG):
                    half, rl = ni % 2, ni // 2
                    ps = psA.tile([16, 512], F32, name='cps', tag='cps')
                    nc.tensor.matmul(
                        ps[:], W73s[:],
                        xt[0:73, ni * 512:(ni + 1) * 512],
                        start=True, stop=True)
                    ps3 = ps[:].rearrange('p (j t) -> p j t', j=8)
                    dst3 = cg4[:, half * 8:(half + 1) * 8, rl, :]
                    if half == 0:
                        nc.scalar.activation(dst3, ps3, AF.Copy)
                    else:
                        nc.vector.tensor_copy(dst3, ps3)
                # scatter group -> convT r-window, then rms for the window
                for kv in range(2):
                    for g in range(2):
                        for c in range(4):
                            row = g * 8 + kv * 4 + c
                            dst = convT[kv][:].rearrange(
                                '(g c j) (r t) -> g c j r t', g=2, c=4, r=RT)
                            nc.gpsimd.dma_start(
                                dst[g, c, :, gi * RG:(gi + 1) * RG, :],
                                cg[row:row + 1, :])
                for kv in range(2):
                    nc.gpsimd.tensor_tensor(sqw[kv][:, gsl],
                                            convT[kv][:, gsl],
                                            convT[kv][:, gsl], OP.mult)
                    ss2 = psB.tile([128, RG * T], F32, name='ss2', tag='acc')
                    for rl in range(RG):
                        r = gi * RG + rl
                        nc.tensor.matmul(ss2[:, rl * T:(rl + 1) * T], S4s[:],
                                         sqw[kv][:, r * T:(r + 1) * T],
                                         start=True, stop=True)
                    nc.scalar.activation(invP[kv][:, gsl], ss2[:],
                                         AF.Sqrt, bias=epsT[:], scale=0.25)
                    nc.vector.reciprocal(invP[kv][:, gsl], invP[kv][:, gsl])
                    rp = rpk if kv == 0 else rpv
                    nc.vector.scalar_tensor_tensor(
                        nkT[kv][:, gsl], convT[kv][:, gsl], rp[:],
                        invP[kv][:, gsl], OP.mult, OP.mult)
            if debug:
                nc.sync.dma_start(d_nkT[:], nkT[0][:])

            # dense: keys/vals [T, DC] (T on partitions), weights preloaded
            kv_sb = {}
            for kv, (Wsb, bS) in enumerate(((WkS, bkS), (WvS, bvS))):
                ps = psA.tile([T, DC], F32, name='dps', tag='dps', bufs=1)
                for r in range(RT):
                    nc.tensor.matmul(ps[:],
                                     nkT[kv][:, r * T:(r + 1) * T],
                                     Wsb[:, r * DC:(r + 1) * DC],
                                     start=(r == 0), stop=False)
                nc.tensor.matmul(ps[:], o64[:],
                                 bS[:], start=False, stop=True)
                sb = pk.tile([T, DC], BF16, name=f'kv{kv}')
                nc.vector.tensor_copy(sb[:], ps[:])
                kv_sb[kv] = sb
            keys, vals = kv_sb[0], kv_sb[1]
            if debug:
                nc.sync.dma_start(d_keys[:], keys[:])
                nc.sync.dma_start(d_vals[:], vals[:])

            # transpose keys -> keysT chunks [98, 64] x4
            keysT = pk.tile([CQ, NQ * T], BF16, name='keysT')
            for q in range(NQ):
                pt = psT.tile([CQ, T], BF16, name='tps', tag='mmT')
                nc.tensor.transpose(pt[:], keys[:, q * CQ:(q + 1) * CQ],
                                    idnb[0:T, 0:T])
                nc.vector.tensor_copy(keysT[:, q * T:(q + 1) * T], pt[:])

            # scb = bcast(sc), scb2, q2 = vals*scb, scv = scb*vals transposed
            psc = psA.tile([T, DC], F32, name='pscb', tag='dps', bufs=1)
            nc.tensor.matmul(psc[:], o64[:], scS[:],
                             start=True, stop=True)
            scb = pk.tile([T, DC], F32, name='scb')
            nc.vector.tensor_copy(scb[:], psc[:])
            scb2 = pk.tile([T, DC], F32, name='scb2')
            nc.vector.tensor_tensor(scb2[:], scb[:], scb[:], OP.mult)
            q2 = pk.tile([T, DC], BF16, name='q2')
            nc.vector.tensor_tensor(q2[:], vals[:], scb[:], OP.mult)
            P3 = pk.tile([CQ, NQ * 3 * T], BF16, name='P3')
            for q in range(NQ):
                pt = psT.tile([CQ, T], BF16, name='tps', tag='mmT')
                nc.tensor.transpose(pt[:], q2[:, q * CQ:(q + 1) * CQ],
                                    idnb[0:T, 0:T])
                nc.vector.tensor_copy(
                    P3[:, (q * 3 + 1) * T:(q * 3 + 2) * T], pt[:])

            # w2T chunks [98, 512] x4 (PE transposes)
            w2T = pk.tile([CQ, NQ * H], BF16, name='w2T')
            for q in range(NQ):
                for m in range(HT):
                    pt = psT.tile([CQ, 128], BF16, name='t2ps', tag='mmT')
                    nc.tensor.transpose(
                        pt[:], w2S[:, m * DC + q * CQ:
                                   m * DC + (q + 1) * CQ], idnb[:])
                    nc.vector.tensor_copy(
                        w2T[:, q * H + m * 128:q * H + (m + 1) * 128], pt[:])

            # G_K = keys keys^T  (accumulate over chunks)
            pgk = psB.tile([T, T], F32, name='pgk', tag='acc')
            for q in range(NQ):
                nc.tensor.matmul(pgk[:], keysT[:, q * T:(q + 1) * T],
                                 keysT[:, q * T:(q + 1) * T],
                                 start=(q == 0), stop=(q == NQ - 1))
            GK = pk.tile([T, T], BF16, name='GK')
            nc.vector.tensor_copy(GK[:], pgk[:])
            if debug:
                nc.sync.dma_start(d_GK[:], GK[:])

            # z1T partial [H(4x128), T] = w1C^T keysT + b1/8 (one PSUM bank)
            z1ps = psB.tile([128, HT * T], F32, name='z1ps', tag='acc')
            for m in range(HT):
                msl = slice(m * T, (m + 1) * T)
                for q in range(NQ):
                    nc.tensor.matmul(z1ps[:, msl],
                                     w1S[:, q * H + m * 128:
                                         q * H + (m + 1) * 128],
                                     keysT[:, q * T:(q + 1) * T],
                                     start=(q == 0), stop=False)
                nc.tensor.matmul(z1ps[:, msl], b1r8[:, m * 128:(m + 1) * 128],
                                 o64[:], start=False, stop=True)
            z1Tp = pk.tile([128, HT * T], BF16, name='z1Tp')
            nc.vector.tensor_copy(z1Tp[:], z1ps[:])

            # ---- R1: AllReduce [128, 256 z1T cols | 32 GK cols] bf16 ----
            r1i = pd.tile([128, HT * T + 32], BF16, name='r1i')
            r1o = pd.tile([128, HT * T + 32], BF16, name='r1o')
            nc.gpsimd.dma_start(r1i[:, 0:HT * T], z1Tp[:])
            nc.gpsimd.dma_start(
                r1i[:, HT * T:HT * T + 32].rearrange('(p h) c -> p h c',
                                                     h=2),
                GK[:].rearrange('p (h c) -> p h c', h=2))
            nc.gpsimd.collective_compute(
                'AllReduce', OP.add, replica_groups=[list(range(NCORES))],
                ins=[r1i.opt()], outs=[r1o.opt()])

            z1T = pk.tile([128, HT * T], BF16, name='z1T')
            nc.sync.dma_start(z1T[:], r1o[:, 0:HT * T])
            GKg = pk.tile([T, T], BF16, name='GKg')
            nc.sync.dma_start(
                GKg[:].rearrange('p (h c) -> p h c', h=2),
                r1o[:, HT * T:HT * T + 32].rearrange('(p h) c -> p h c',
                                                     h=2))
            if debug:
                for m in range(HT):
                    nc.sync.dma_start(d_z1T[m * 128:(m + 1) * 128, :],
                                      z1T[:, m * T:(m + 1) * T])

            # R64 = diag(wv) @ (GK + 1)  (for z1f correction incl. agg_b1)
            R64 = pk.tile([T, T], BF16, name='R64')
            nc.vector.tensor_scalar(R64[:], GKg[:], 1.0, wvS[:],
                                    OP.add, OP.mult)

            # =========== PHASE 2 ===========
            hT = pk.tile([128, HT * T], BF16, name='hT')
            nc.scalar.activation(hT[:], z1T[:], AF.Gelu_apprx_tanh)
            # h [T, H]
            h = pk.tile([T, H], BF16, name='h')
            for m in range(HT):
                pt = psT.tile([T, 128], BF16, name='hps', tag='mmT')
                nc.tensor.transpose(pt[:], hT[:, m * T:(m + 1) * T], idnb[:])
                nc.vector.tensor_copy(h[:, m * 128:(m + 1) * 128], pt[:])

            # y = h @ w2C  [T, DC]
            py = psA.tile([T, DC], F32, name='py', tag='dps', bufs=1)
            for m in range(HT):
                nc.tensor.matmul(py[:], hT[:, m * T:(m + 1) * T],
                                 w2S[:, m * DC:(m + 1) * DC],
                                 start=(m == 0), stop=(m == HT - 1))
            y = pk.tile([T, DC], BF16, name='y')
            nc.vector.tensor_copy(y[:], py[:])
            if debug:
                nc.sync.dma_start(d_y[:], y[:])

            # yT chunks + (sc^2 y)T into P3 slots (i=2: yT, i=0: s2yT)
            for q in range(NQ):
                ysl = slice((q * 3 + 2) * T, (q * 3 + 3) * T)
                pt = psT.tile([CQ, T], BF16, name='tps', tag='mmT')
                nc.tensor.transpose(pt[:], y[:, q * CQ:(q + 1) * CQ],
                                    idnb[0:T, 0:T])
                nc.vector.tensor_copy(P3[:, ysl], pt[:])
                nc.vector.tensor_scalar(P3[:, (q * 3) * T:(q * 3 + 1) * T],
                                        P3[:, ysl],
                                        scsqTS[:, q:q + 1], None,
                                        OP.mult)

            # scalars C = sum y^2, A = sum (scb y)^2, B = sum (scb y) v
            ua = pk.tile([T, DC], F32, name='ua')
            nc.vector.tensor_tensor(ua[:], y[:], scb[:], OP.mult)
            scr = pk.tile([T, DC], F32, name='scr')
            Cc = pk.tile([T, 1], F32, name='Cc')
            Ac = pk.tile([T, 1], F32, name='Ac')
            Bc = pk.tile([T, 1], F32, name='Bc')
            nc.scalar.activation(scr[:], y[:], AF.Square, accum_out=Cc[:])
            nc.scalar.activation(scr[:], ua[:], AF.Square, accum_out=Ac[:])
            nc.vector.scalar_tensor_tensor(scr[:], ua[:], 1.0, vals[:],
                                           OP.mult, OP.mult,
                                           accum_out=Bc[:])

            # P matmuls: out block (m) = [P1m | P2m | P3m], shared stationary
            Pt = pk.tile([128, 3 * HT * T], BF16, name='Pt')
            for m in range(HT):
                pp = psB.tile([128, 3 * T], F32, name='pp', tag='acc')
                for q in range(NQ):
                    nc.tensor.matmul(
                        pp[:],
                        w2T[:, q * H + m * 128:q * H + (m + 1) * 128],
                        P3[:, q * 3 * T:(q + 1) * 3 * T],
                        start=(q == 0), stop=(q == NQ - 1))
                nc.vector.tensor_copy(
                    Pt[:, m * 3 * T:(m + 1) * 3 * T], pp[:])

            # ---- R2: AllReduce [128, 768 P cols | 3 C/A/B cols] bf16 ----
            NP2 = 3 * HT * T
            r2i = pd.tile([128, NP2 + 3], BF16, name='r2i')
            r2o = pd.tile([128, NP2 + 3], BF16, name='r2o')
            nc.gpsimd.dma_start(r2i[:, 0:NP2], Pt[:])
            nc.gpsimd.dma_start(r2i[0:T, NP2 + 0:NP2 + 1], Cc[:])
            nc.gpsimd.dma_start(r2i[0:T, NP2 + 1:NP2 + 2], Ac[:])
            nc.gpsimd.dma_start(r2i[0:T, NP2 + 2:NP2 + 3], Bc[:])
            nc.gpsimd.collective_compute(
                'AllReduce', OP.add, replica_groups=[list(range(NCORES))],
                ins=[r2i.opt()], outs=[r2o.opt()])

            Pg = pk.tile([128, 3 * HT * T], BF16, name='Pg')
            nc.sync.dma_start(Pg[:], r2o[:, 0:NP2])
            CAB = pk.tile([T, 3], BF16, name='CAB')
            nc.sync.dma_start(CAB[:], r2o[0:T, NP2:NP2 + 3])

            # scalar chain, column space [T, 1] (cols of scol: inv a1 a2 a3)
            scol = pk.tile([T, 4], F32, name='scol')
            i2c = pk.tile([T, 1], F32, name='i2c')
            t1c = pk.tile([T, 1], F32, name='t1c')
            Scc = pk.tile([T, 1], F32, name='Scc')
            nc.scalar.activation(scol[:, 0:1], CAB[:, 0:1], AF.Sqrt,
                                 bias=epsT[0:T, :], scale=1.0 / D)
            nc.vector.reciprocal(scol[:, 0:1], scol[:, 0:1])
            nc.vector.tensor_tensor(i2c[:], scol[:, 0:1], scol[:, 0:1],
                                    OP.mult)
            # S = 2 inv A - 2 B
            nc.vector.scalar_tensor_tensor(t1c[:], scol[:, 0:1], 2.0,
                                           CAB[:, 1:2], OP.mult, OP.mult)
            nc.vector.scalar_tensor_tensor(Scc[:], CAB[:, 2:3], -2.0,
                                           t1c[:], OP.mult, OP.add)
            # a1 = 2 inv^2 ; a2 = 2 inv ; a3 = inv^3 S / D
            nc.vector.tensor_scalar(scol[:, 1:2], i2c[:], 2.0, None, OP.mult)
            nc.vector.tensor_scalar(scol[:, 2:3], scol[:, 0:1], 2.0, None,
                                    OP.mult)
            nc.vector.scalar_tensor_tensor(t1c[:], i2c[:], 1.0 / D,
                                           scol[:, 0:1], OP.mult, OP.mult)
            nc.vector.tensor_tensor(scol[:, 3:4], t1c[:], Scc[:], OP.mult)

            # a1/a2/a3 to a single [1, 3T] row, then broadcast ab2 [128, 3T]
            r3ps = psB.tile([1, 3 * T], F32, name='r3ps', tag='mm64', bufs=1)
            for j in range(3):
                nc.tensor.matmul(r3ps[:, j * T:(j + 1) * T],
                                 scol[:, 1 + j:2 + j], idn[0:T, 0:T],
                                 start=True, stop=True)
            arow = pk.tile([1, 3 * T], F32, name='arow')
            nc.vector.tensor_copy(arow[:], r3ps[:])
            abps = psB.tile([128, 3 * T], F32, name='abps', tag='mm64',
                            bufs=1)
            for j in range(3):
                nc.tensor.matmul(abps[:, j * T:(j + 1) * T], o128[:],
                                 arow[:, j * T:(j + 1) * T],
                                 start=True, stop=True)
            ab2 = pk.tile([128, 3 * T], F32, name='ab2')
            nc.vector.tensor_copy(ab2[:], abps[:])

            # dhT = a1*P1 - a2*P2 - a3*P3 ; dz1T = dhT * gelu'(z1T)
            dgel = pk.tile([128, HT * T], BF16, name='dgel')
            nc.scalar.activation(dgel[:], z1T[:], AF.Derivative_Gelu)
            dhT = pk.tile([128, HT * T], F32, name='dhT')
            tmpA = pk.tile([128, 3 * T], F32, name='tmpA')
            for m in range(HT):
                msl = slice(m * T, (m + 1) * T)
                nc.vector.tensor_tensor(tmpA[:],
                                        Pg[:, m * 3 * T:(m + 1) * 3 * T],
                                        ab2[:], OP.mult)
                nc.vector.tensor_tensor(dhT[:, msl], tmpA[:, 0:T],
                                        tmpA[:, T:2 * T], OP.subtract)
                nc.vector.tensor_tensor(dhT[:, msl], dhT[:, msl],
                                        tmpA[:, 2 * T:3 * T], OP.subtract)
            dz1T = pk.tile([128, HT * T], BF16, name='dz1T')
            nc.vector.tensor_tensor(dz1T[:], dhT[:], dgel[:], OP.mult)

            # dz1 [T, H]
            dz1 = pk.tile([T, H], BF16, name='dz1')
            for m in range(HT):
                pt = psT.tile([T, 128], BF16, name='dzps', tag='mmT')
                nc.tensor.transpose(pt[:], dz1T[:, m * T:(m + 1) * T], idnb[:])
                nc.vector.tensor_copy(dz1[:, m * 128:(m + 1) * 128], pt[:])

            # z1fT = z1T - dz1^T-weighted: T2T[m] = dz1[:,m]^T @ R64
            t2ps = psB.tile([128, HT * T], F32, name='t2t', tag='mm64',
                            bufs=1)
            for m in range(HT):
                nc.tensor.matmul(t2ps[:, m * T:(m + 1) * T],
                                 dz1[:, m * 128:(m + 1) * 128],
                                 R64[:], start=True, stop=True)
            z1fT = pk.tile([128, HT * T], F32, name='z1fT')
            nc.vector.tensor_tensor(z1fT[:], z1T[:], t2ps[:], OP.subtract)
            hfT = pk.tile([128, HT * T], BF16, name='hfT')
            nc.scalar.activation(hfT[:], z1fT[:], AF.Gelu_apprx_tanh)

            # G = a1*(scb2*y) - a2*(q2) - a3*y  (column scalars)
            G = pk.tile([T, DC], F32, name='G')
            gt1 = pk.tile([T, DC], F32, name='gt1')
            nc.vector.tensor_tensor(gt1[:], y[:], scb2[:], OP.mult)
            nc.vector.tensor_scalar(G[:], gt1[:], scol[:, 1:2], None, OP.mult)
            nc.vector.tensor_scalar(gt1[:], q2[:], scol[:, 2:3], None, OP.mult)
            nc.vector.tensor_tensor(G[:], G[:], gt1[:], OP.subtract)
            nc.vector.tensor_scalar(gt1[:], y[:], scol[:, 3:4], None, OP.mult)
            nc.vector.tensor_tensor(G[:], G[:], gt1[:], OP.subtract)

            # agg_w2 & w2' = w2 - h^T (wv*G)
            wG = pk.tile([T, DC], BF16, name='wG')
            nc.vector.tensor_scalar(wG[:], G[:], wvS[:], None, OP.mult)
            w2p = pk.tile([128, HT * DC], BF16, name='w2p')
            for m in range(HT):
                pa = psA.tile([128, DC], F32, name='paw2', tag='dps', bufs=1)
                nc.tensor.matmul(pa[:],
                                 h[:, m * 128:(m + 1) * 128],
                                 wG[:], start=True, stop=True)
                nc.vector.tensor_tensor(w2p[:, m * DC:(m + 1) * DC],
                                        w2S[:, m * DC:(m + 1) * DC], pa[:],
                                        OP.subtract)
                if debug:
                    nc.sync.dma_start(d_w2p[m * 128:(m + 1) * 128, :],
                                      w2p[:, m * DC:(m + 1) * DC])

            # rows: b2' ; sc' ; sc'*ros (all partition-0 tiles)
            brow = pk.tile([1, 3 * DC], F32, name='brow')
            pr = psB.tile([1, DC], F32, name='prow', tag='acc')
            nc.tensor.matmul(pr[:], wvS[:], G[:],
                             start=True, stop=True)
            nc.vector.tensor_tensor(brow[:, 0:DC], b2S[:], pr[:], OP.subtract)

            # r2y = 2*inv*(scb*y)*y - 2*v*y ; agg_sc = (wv*inv)^T r2y
            nc.vector.tensor_tensor(gt1[:], ua[:], y[:], OP.mult)
            nc.vector.tensor_scalar(gt1[:], gt1[:], scol[:, 2:3], None, OP.mult)
            r2y2 = pk.tile([T, DC], F32, name='r2y2')
            nc.vector.tensor_tensor(r2y2[:], vals[:], y[:], OP.mult)
            nc.vector.tensor_scalar(r2y2[:], r2y2[:], 2.0, None, OP.mult)
            nc.vector.tensor_tensor(gt1[:], gt1[:], r2y2[:], OP.subtract)
            wiv = pk.tile([T, 1], F32, name='wiv')
            nc.vector.tensor_tensor(wiv[:], wvS[:], scol[:, 0:1], OP.mult)
            pr2 = psB.tile([1, DC], F32, name='prow2', tag='acc')
            nc.tensor.matmul(pr2[:], wiv[:],
                             gt1[:], start=True, stop=True)
            nc.vector.tensor_tensor(brow[:, DC:2 * DC], scS32[:], pr2[:],
                                    OP.subtract)
            nc.vector.tensor_tensor(brow[:, 2 * DC:3 * DC],
                                    brow[:, DC:2 * DC], rosS[:], OP.mult)

            # transpose rows to columns: colrows[:, q*3+j]
            colrows = pk.tile([CQ, NQ * 3], F32, name='colrows')
            for q in range(NQ):
                pt = psB.tile([CQ, 3], F32, name='crps', tag='mm64', bufs=1)
                for j in range(3):
                    nc.tensor.transpose(
                        pt[:, j:j + 1],
                        brow[:, j * DC + q * CQ:j * DC + (q + 1) * CQ],
                        idn[0:1, 0:1])
                nc.vector.tensor_copy(colrows[:, q * 3:(q + 1) * 3], pt[:])

            # yfT chunks [98, T] = w2p^T @ hfT + b2'T ; squares and partials
            yfT = pk.tile([CQ, NQ * T], F32, name='yfT')
            sqf = pk.tile([CQ, NQ * T], BF16, name='sqf')
            ssqf = pk.tile([CQ, NQ * T], BF16, name='ssqf')
            for q in range(NQ):
                pf = psB.tile([CQ, T], F32, name='pyf', tag='acc')
                for m in range(HT):
                    nc.tensor.matmul(pf[:],
                                     w2p[:, m * DC + q * CQ:m * DC + (q + 1) * CQ],
                                     hfT[:, m * T:(m + 1) * T],
                                     start=(m == 0), stop=(m == HT - 1))
                sl = slice(q * T, (q + 1) * T)
                nc.vector.tensor_scalar(yfT[:, sl], pf[:],
                                        colrows[:, q * 3:q * 3 + 1], None,
                                        OP.add)
                nc.vector.tensor_tensor(sqf[:, sl], yfT[:, sl], yfT[:, sl],
                                        OP.mult)
                nc.vector.tensor_scalar(ssqf[:, sl], yfT[:, sl],
                                        colrows[:, q * 3 + 1:q * 3 + 2], None,
                                        OP.mult)
                nc.vector.tensor_tensor(ssqf[:, sl], ssqf[:, sl], ssqf[:, sl],
                                        OP.mult)
            if debug:
                for q in range(NQ):
                    nc.sync.dma_start(d_yfT[q * CQ:(q + 1) * CQ, :],
                                      yfT[:, q * T:(q + 1) * T])
            pfin = psB.tile([1, 2 * T], F32, name='pfin', tag='acc')
            for q in range(NQ):
                nc.tensor.matmul(pfin[:, 0:T], ocol[0:CQ, :],
                                 sqf[:, q * T:(q + 1) * T],
                                 start=(q == 0), stop=(q == NQ - 1))
            for q in range(NQ):
                nc.tensor.matmul(pfin[:, T:2 * T], ocol[0:CQ, :],
                                 ssqf[:, q * T:(q + 1) * T],
                                 start=(q == 0), stop=(q == NQ - 1))
            fin = pk.tile([1, 2 * T], F32, name='fin')
            nc.vector.tensor_copy(fin[:], pfin[:])

            # ---- R3: AllReduce final scalars ----
            r3i = pd.tile([1, 2 * T], F32, name='r3i')
            r3o = pd.tile([1, 2 * T], F32, name='r3o')
            nc.gpsimd.dma_start(r3i[:], fin[:])
            nc.gpsimd.collective_compute(
                'AllReduce', OP.add, replica_groups=[list(range(NCORES))],
                ins=[r3i.opt()], outs=[r3o.opt()])

            # invf = rsqrt(Cf/D + eps); invp = rsqrt(invf^2 * Af/D + eps)
            CfAf = pk.tile([1, 2 * T], F32, name='CfAf')
            nc.sync.dma_start(CfAf[:], r3o[:])
            invft = pk.tile([1, T], F32, name='invft')
            invpt = pk.tile([1, T], F32, name='invpt')
            fft = pk.tile([1, T], F32, name='fft')
            nc.scalar.activation(invft[:], CfAf[:, 0:T], AF.Sqrt,
                                 bias=epsT[0:1, :], scale=1.0 / D)
            nc.vector.reciprocal(invft[:], invft[:])
            nc.vector.tensor_tensor(invpt[:], invft[:], invft[:], OP.mult)
            nc.vector.tensor_tensor(invpt[:], invpt[:], CfAf[:, T:2 * T],
                                    OP.mult)
            nc.scalar.activation(invpt[:], invpt[:], AF.Sqrt,
                                 bias=epsT[0:1, :], scale=1.0 / D)
            nc.vector.reciprocal(invpt[:], invpt[:])
            nc.vector.tensor_tensor(fft[:], invft[:], invpt[:], OP.mult)
            ffb = pk.tile([128, T], F32, name='ffb')
            pt = psB.tile([128, T], F32, name='ffps', tag='mm64', bufs=1)
            nc.tensor.matmul(pt[:], o128[:], fft[:], start=True, stop=True)
            nc.vector.tensor_copy(ffb[:], pt[:])

            # out = yfT * scrosT * ff  (single [CQ, NQ*T] store)
            outsb = pk.tile([CQ, NQ * T], F32, name='outsb')
            for q in range(NQ):
                sl = slice(q * T, (q + 1) * T)
                nc.vector.scalar_tensor_tensor(
                    outsb[:, sl], yfT[:, sl],
                    colrows[:, q * 3 + 2:q * 3 + 3], ffb[0:CQ, :],
                    OP.mult, OP.mult)
            nc.sync.dma_start(out[:], outsb[:])

    _split_excess_waits(nc)
    return nc, sorted(dbg_outs.keys())


def make_inputs(inputs):
    """Build the 8 per-core input dicts from the full problem inputs."""
    x = np.asarray(inputs['x'], np.float32)
    x_t = np.transpose(x, (0, 2, 3, 1))
    X72 = _build_im2col(x_t)
    W73 = _build_w73(np.asarray(inputs['conv_k_w'], np.float32),
                     np.asarray(inputs['conv_k_b'], np.float32),
                     np.asarray(inputs['conv_v_w'], np.float32),
                     np.asarray(inputs['conv_v_b'], np.float32))
    perm, valid = _din_perm()
    dkw = np.asarray(inputs['dense_k_w'], np.float32)
    dvw = np.asarray(inputs['dense_v_w'], np.float32)
    Wk_full = np.zeros((DINP, D), np.float32)
    Wv_full = np.zeros((DINP, D), np.float32)
    Wk_full[valid] = dkw[perm[valid]]
    Wv_full[valid] = dvw[perm[valid]]

    w1 = np.asarray(inputs['mem_w1'], np.float32)
    w2 = np.asarray(inputs['mem_w2'], np.float32)
    sc = np.asarray(inputs['mem_scale'], np.float32)
    ros = np.asarray(inputs['rms_out_scale'], np.float32)
    dkb = np.asarray(inputs['dense_k_b'], np.float32)
    dvb = np.asarray(inputs['dense_v_b'], np.float32)
    b1 = np.asarray(inputs['mem_b1'], np.float32)
    b2 = np.asarray(inputs['mem_b2'], np.float32)

    base = {
        'X80': _pack_slabs(X72).astype(NPBF),
    }
    CE = np.zeros((128, 144), np.float32)
    CE[:, 0:128] = _s4()
    CE[0:73, 128:144] = W73
    base['CE'] = CE.astype(NPBF)
    CF = np.zeros((128, 2), np.float32)
    CF[:, 0:1] = _rms_pattern(np.asarray(inputs['rms_k_scale'], np.float32))
    CF[:, 1:2] = _rms_pattern(np.asarray(inputs['rms_v_scale'], np.float32))
    base['CF'] = CF

    in_maps = []
    for c in range(NCORES):
        sl = slice(c * DC, (c + 1) * DC)
        m = dict(base)
        CA = np.zeros(3 * DC + T + H, np.float32)
        CA[0:DC] = dkb[sl]
        CA[DC:2 * DC] = dvb[sl]
        CA[2 * DC:3 * DC] = sc[sl]
        CA[3 * DC:3 * DC + T] = 1.0
        CA[3 * DC + T:] = b1 / NCORES
        m['CA'] = CA.reshape(1, -1).astype(NPBF)
        CB = np.zeros(3 * DC + 128, np.float32)
        CB[0:DC] = b2[sl]
        CB[DC:2 * DC] = sc[sl]
        CB[2 * DC:3 * DC] = ros[sl]
        CB[3 * DC:] = 1.0
        m['CB'] = CB.reshape(1, -1)
        CD = np.zeros((128, 133), np.float32)
        CD[:, 0:128] = np.eye(128)
        CD[0:CQ, 128:132] = (sc[sl] ** 2).reshape(NQ, CQ).T
        CD[0:T, 132] = _wvec()
        m['CD'] = CD
        CC = np.zeros((128, 129 + HT * DC + NQ * H + 2 * RT * DC),
                      np.float32)
        CC[:, 0:128] = np.eye(128)
        CC[:, 128] = 1.0
        o = 129
        w2c = w2[:, sl]
        CC[:, o:o + HT * DC] = (w2c.reshape(HT, 128, DC).transpose(1, 0, 2)
                                .reshape(128, HT * DC))
        o += HT * DC
        w1c = w1[sl, :]
        CC[0:CQ, o:o + NQ * H] = (w1c.reshape(NQ, CQ, H).transpose(1, 0, 2)
                                  .reshape(CQ, NQ * H))
        o += NQ * H
        CC[:, o:o + RT * DC] = (Wk_full[:, sl].reshape(RT, 128, DC)
                                .transpose(1, 0, 2).reshape(128, RT * DC))
        o += RT * DC
        CC[:, o:o + RT * DC] = (Wv_full[:, sl].reshape(RT, 128, DC)
                                .transpose(1, 0, 2).reshape(128, RT * DC))
        m['CC'] = CC.astype(NPBF)
        in_maps.append(m)
    return in_maps


def kernel(**inputs):
    if 'nc' not in _NC_CACHE:
        _NC_CACHE['nc'], _ = build_nc(debug=False)
    nc = _NC_CACHE['nc']
    in_maps = make_inputs(inputs)
    res = run_bass_kernel_spmd(nc, in_maps, list(range(NCORES)))
    blocks = [res.results[c]['out'].reshape(CQ, NQ, T).transpose(1, 0, 2)
              .reshape(DC, T) for c in range(NCORES)]
    YT = np.concatenate(blocks, axis=0)
    return np.ascontiguousarray(YT.T).reshape(T, 4, 28, 28)



# revision 48
# speedup vs baseline: 1.1252x; 1.0915x over previous
"""Trainium2 Bass kernel for nn_MirasModel (scatter_memory).

Strategy (8 NeuronCores, SPMD):
  - Column-shard the shared D=3136 feature dimension: core c owns Dc=392
    columns of dense_k_w / dense_v_w / mem_w2 / biases / scales, and the
    matching 392 rows of mem_w1.
  - Conv + rmsnorm computed fully on every core (tiny) via a packed
    im2col matmul, with a DMA scatter producing the transposed
    [Din, T] activation layout the dense matmuls need.
  - Three AllReduce rounds:
      R1: z1 = keys@w1+b1 partial sums  +  Gram(keys) = keys keys^T
      R2: per-token scalars (C,A,B) + backward projections P1,P2,P3
      R3: final-forward rmsnorm scalar partials
    The Gram matrix lets z1f = z1 - G_K diag(w) dz1 be computed locally,
    eliminating a fourth round (keys @ agg_w1 == Gram @ diag(w) @ dz1).
  - All heavy DMA (im2col + dense weight shards) hides under R1's
    collective entry latency.
"""

import sys

if '/opt/trn_rl_repo' not in sys.path:
    sys.path.insert(0, '/opt/trn_rl_repo')

import numpy as np

import concourse.bass as bass
import concourse.mybir as mybir
from concourse import tile
from concourse.bass_utils import run_bass_kernel_spmd

F32 = mybir.dt.float32
F32R = mybir.dt.float32r
BF16 = mybir.dt.bfloat16
NPBF = mybir.dt.np(mybir.dt.bfloat16)
AF = mybir.ActivationFunctionType
OP = mybir.AluOpType

T = 64
D = 3136
H = 512
NCORES = 8
DC = D // NCORES            # 392 columns per core
CQ = 98                     # Dc sub-chunk (4 per core)
NQ = DC // CQ               # 4
PPIX = 800                  # padded pixel count (784 real + 16 dummy)
DINP = PPIX * 4             # padded Din = 3200
RT = DINP // 128            # 25 Din tiles
NPTR = PPIX // 2            # 400 pixel-pairs
NCONV = NPTR * T // 512     # 50 conv matmul chunks
CPAIR = 10                  # conv chunks per DMA slab (one r-group)
NSLAB = NCONV // CPAIR      # 25 slabs
XROWS = 80                  # 73 im2col rows padded to 80 (16 | 80*512)
HT = H // 128               # 4 H tiles
ALPHA, ETA0, EPS = 0.9, 0.1, 1e-6

_NC_CACHE = {}


# ---------------------------------------------------------------------------
# walrus workaround: this compiler build rejects Drain instructions carrying
# more than one sync wait; split extras onto preceding Drains.
def _split_excess_waits(nc):
    """This walrus build has tight per-instruction sync-wait budgets
    (1 for Drain/Matmult/etc).  Move excess waits onto preceding NoOps."""
    LIM1 = 1

    def limit_for(ins):
        return LIM1

    n_new = 0
    for fn in nc.m.functions:
        for bb in fn.blocks:
            i = 0
            while i < len(bb.instructions):
                ins = bb.instructions[i]
                si = getattr(ins, 'sync_info', None)
                lim = limit_for(ins)
                if (si is not None and si.on_wait and len(si.on_wait) > lim
                        and getattr(ins, 'engine', None) is not None):
                    waits = list(si.on_wait)
                    keep, extra = waits[:lim], waits[lim:]
                    ins.sync_info = mybir.SyncInfo(on_wait=keep,
                                                  on_update=si.on_update)
                    pos = i
                    for j in range(0, len(extra), LIM1):
                        n_new += 1
                        nd = mybir.InstNoOp(
                            name=f"I-waitfix-{n_new}",
                            engine=ins.engine,
                            bass_nofuse=True,
                            sync_info=mybir.SyncInfo(
                                on_wait=extra[j:j + LIM1], on_update=[]),
                        )
                        bb.instructions.insert(pos, nd)
                        pos += 1
                        i += 1
                i += 1
    return n_new


def _din_perm():
    """Device Din row -> reference Din index (p*4+c), p,c of padded grid."""
    idx = np.zeros(DINP, np.int64)
    valid = np.zeros(DINP, bool)
    for r in range(RT):
        for i in range(128):
            g, c, jj = i // 64, (i % 64) // 16, i % 16
            p = 2 * (16 * r + jj) + g
            row = r * 128 + i
            if p < 784:
                idx[row] = p * 4 + c
                valid[row] = True
    return idx, valid


def _pack_slabs(X72):
    """Repack [73, NCONV*512] im2col into DMA-slab layout.

    Returns [NSLAB*XROWS, CPAIR*512]; slab s rows 0:73 = X72 cols
    s*CPAIR*512:(s+1)*CPAIR*512, rows 73:80 zero.  Row-contiguous slabs
    let the HWDGE spray descriptors across all 16 DMA engines (a
    strided source pins the whole transfer to one engine).
    """
    Xs = np.zeros((NSLAB * XROWS, CPAIR * 512), np.float32)
    v = X72.reshape(73, NSLAB, CPAIR * 512)
    for s in range(NSLAB):
        Xs[s * XROWS:s * XROWS + 73, :] = v[:, s, :]
    return Xs


def _build_im2col(x_t, pad_val=0.0):
    """x_t: (T, 28, 28, 4) NHWC.  Returns X72 [73, NPTR*64] fp32.

    row = g*36 + (di*3+dj)*4 + ci  (g in 0..1), row 72 = ones.
    col = ptr*64 + t, pixel p = 2*ptr + g (row-major 28x28, padded to 800).
    """
    xp = np.zeros((T, 30, 30, 4), np.float32)
    xp[:, 1:29, 1:29, :] = x_t
    X = np.zeros((73, NPTR * T), np.float32)
    p = np.arange(PPIX)
    pi, pj = p // 28, p % 28
    ok = p < 784
    for g in range(2):
        psel = p[(p % 2) == g]
        ptr = psel // 2
        pis, pjs, oks = pi[(p % 2) == g], pj[(p % 2) == g], ok[(p % 2) == g]
        for di in range(3):
            for dj in range(3):
                for ci in range(4):
                    row = g * 36 + (di * 3 + dj) * 4 + ci
                    vals = np.zeros((NPTR, T), np.float32)
                    vsel = xp[:, np.clip(pis + di, 0, 29),
                              np.clip(pjs + dj, 0, 29), ci]  # (T, NPTR)
                    vals[oks[: NPTR], :] = vsel.T[oks[: NPTR], :]
                    # dummy pixels (>=784) contribute garbage later discarded
                    X[row, :] = vals.reshape(-1)
    X[72, :] = 1.0
    return X


def _build_w73(conv_k_w, conv_k_b, conv_v_w, conv_v_b):
    """W73 [73, 16]; col = g*8 + kv*4 + co."""
    W = np.zeros((73, 16), np.float32)
    for g in range(2):
        for kv, (w, b) in enumerate(((conv_k_w, conv_k_b),
                                     (conv_v_w, conv_v_b))):
            for di in range(3):
                for dj in range(3):
                    for ci in range(4):
                        W[g * 36 + (di * 3 + dj) * 4 + ci,
                          g * 8 + kv * 4:g * 8 + kv * 4 + 4] = w[di, dj, ci, :]
            W[72, g * 8 + kv * 4:g * 8 + kv * 4 + 4] = b
    return W


def _rms_pattern(scale4):
    """[128,1] per-partition rms scale: partition i -> scale4[(i%64)//16]."""
    i = np.arange(128)
    return scale4[(i % 64) // 16].astype(np.float32).reshape(128, 1)


def _s4():
    """Dup-selector [128, 128]: S[i, o] = 1 iff (g, j) of i == (g, j) of o.

    Partition layout (g, c, j): g = i // 64, c = (i % 64) // 16, j = i % 16.
    The sumsq matmul with this stationary yields the per-pixel channel
    sum-of-squares already duplicated across the 4 c-slots."""
    i = np.arange(128)
    gj = (i // 64) * 16 + (i % 16)
    return (gj[:, None] == gj[None, :]).astype(np.float32)


def _wvec():
    betas = (np.float32(ALPHA) ** np.arange(T, dtype=np.float32)).astype(np.float32)
    etas = (np.float32(ETA0) * betas).astype(np.float32)
    weights = (etas * (betas[-1] / betas)).astype(np.float32)
    return (np.float32(1e-4) * weights).astype(np.float32)


def build_nc(debug=False):
    nc = bass.Bass()

    def inp(name, shape, dt=F32):
        return nc.dram_tensor(name, list(shape), dt, kind="ExternalInput")

    X80 = inp('X80', (NSLAB * XROWS, CPAIR * 512), BF16)
    # combo tensors (host-packed):
    #   CE (bf16 [128, 144]): S4 dup-selector | W73
    #   CF (f32 [128, 2]): rmspk | rmspv
    #   CA (bf16 [1, 1752]): bk | bv | sc | o64 | b1/8
    #   CB (f32 [1, 1304]): b2 | sc32 | ros | o128
    #   CC (bf16 [128, 23345]): idnb | ocol | w2 | w1(pad128) | Wk | Wv
    #   CD (f32 [128, 133]): idn | scsqT | wv
    CE = inp('CE', (128, 144), BF16)
    CF = inp('CF', (128, 2))
    CA = inp('CA', (1, 3 * DC + T + H), BF16)
    CB = inp('CB', (1, 3 * DC + 128))
    CD = inp('CD', (128, 133))
    CC = inp('CC', (128, 129 + HT * DC + NQ * H + 2 * RT * DC), BF16)

    out = nc.dram_tensor('out', [CQ, NQ * T], F32, kind="ExternalOutput")
    dbg_outs = {}

    def dbg(name, shape):
        if debug:
            dbg_outs[name] = nc.dram_tensor(name, list(shape), F32,
                                            kind="ExternalOutput")
        return dbg_outs.get(name)

    d_nkT = dbg('d_nkT', (128, RT * T))
    d_keys = dbg('d_keys', (T, DC))
    d_vals = dbg('d_vals', (T, DC))
    d_z1T = dbg('d_z1T', (H, T))
    d_GK = dbg('d_GK', (T, T))
    d_y = dbg('d_y', (T, DC))
    d_P = dbg('d_P', (3 * H, T))
    d_dhT = dbg('d_dhT', (H, T))
    d_z1fT = dbg('d_z1fT', (H, T))
    d_w2p = dbg('d_w2p', (H, DC))
    d_yfT = dbg('d_yfT', (DC, T))

    with tile.TileContext(nc) as tc:
        with (
            tc.tile_pool(name='consts', bufs=1) as pc,
            tc.tile_pool(name='wstream', bufs=4) as pw,
            tc.tile_pool(name='xstream', bufs=4) as px,
            tc.tile_pool(name='big', bufs=1) as pb,
            tc.tile_pool(name='work', bufs=1) as pk,
            tc.tile_pool(name='psA', bufs=2, space='PSUM') as psA,
            tc.tile_pool(name='psB', bufs=2, space='PSUM') as psB,
            tc.tile_pool(name='psT', bufs=1, space='PSUM') as psT,
            tc.tile_pool(name='dram', bufs=1, space='DRAM') as pd,
        ):
            # ---- constants to SBUF: 2 sync + 4 scalar-queue loads ----
            CEs = pc.tile([128, 144], BF16, name='CEs')
            nc.sync.dma_start(CEs[:], CE[:])
            CFs = pc.tile([128, 2], F32, name='CFs')
            nc.sync.dma_start(CFs[:], CF[:])
            CAs = pc.tile([1, 3 * DC + T + H], BF16, name='CAs')
            nc.scalar.dma_start(CAs[:], CA[:])
            CBs = pc.tile([1, 3 * DC + 128], F32, name='CBs')
            nc.scalar.dma_start(CBs[:], CB[:])
            CDs = pc.tile([128, 133], F32, name='CDs')
            nc.scalar.dma_start(CDs[:], CD[:])
            CCs = pc.tile([128, 129 + HT * DC + NQ * H + 2 * RT * DC], BF16,
                          name='CCs')
            nc.scalar.dma_start(CCs[:], CC[:])

            S4s = CEs[:, 0:128]
            W73s = CEs[0:73, 128:144]
            rpk = CFs[:, 0:1]
            rpv = CFs[:, 1:2]
            bkS = CAs[:, 0:DC]
            bvS = CAs[:, DC:2 * DC]
            scS = CAs[:, 2 * DC:3 * DC]
            o64 = CAs[:, 3 * DC:3 * DC + T]
            b1r8 = CAs[:, 3 * DC + T:3 * DC + T + H]
            b2S = CBs[:, 0:DC]
            scS32 = CBs[:, DC:2 * DC]
            rosS = CBs[:, 2 * DC:3 * DC]
            o128 = CBs[:, 3 * DC:3 * DC + 128]
            idn = CDs[:, 0:128]
            scsqTS = CDs[0:CQ, 128:132]
            wvS = CDs[0:T, 132:133]
            idnb = CCs[:, 0:128]
            ocol = CCs[:, 128:129]
            OW2 = 129
            OW1 = OW2 + HT * DC
            OWK = OW1 + NQ * H
            OWV = OWK + RT * DC
            w2S = CCs[:, OW2:OW2 + HT * DC]
            w1S = CCs[0:CQ, OW1:OW1 + NQ * H]
            WkS = CCs[:, OWK:OWK + RT * DC]
            WvS = CCs[:, OWV:OWV + RT * DC]
            epsT = pc.tile([128, 1], F32, name='epsT')
            nc.gpsimd.memset(epsT[:], EPS)

            # =========== PHASE 1 ===========
            # conv: 50 chunks of 512 cols; output rows 16 = (g, kv, c)
            # staged in 5 groups of 10 chunks to bound SBUF usage
            convT = {0: pb.tile([128, RT * T], BF16, name='convT0'),
                     1: pb.tile([128, RT * T], BF16, name='convT1')}
            sqw = {0: pb.tile([128, RT * T], BF16, name='sqw0'),
                   1: pb.tile([128, RT * T], BF16, name='sqw1')}
            invP = {0: pb.tile([128, RT * T], F32, name='invP0'),
                    1: pb.tile([128, RT * T], F32, name='invP1')}
            nkT = {0: pb.tile([128, RT * T], BF16, name='nkT0'),
                   1: pb.tile([128, RT * T], BF16, name='nkT1')}
            RG = 5                      # r-tiles per slab (== CPAIR/2)
            for gi in range(RT // RG):
                gsl = slice(gi * RG * T, (gi + 1) * RG * T)
                cg = px.tile([16, 16 * RG * T], BF16, name='cg', tag='cg',
                             bufs=2)
                cg4 = cg[:].rearrange('p (j r t) -> p j r t', j=16, r=RG)
                xt = px.tile([XROWS, CPAIR * 512], BF16, name='xch',
                             tag='xch', bufs=2)
                nc.sync.dma_start(xt[:],
                                  X80[gi * XROWS:(gi + 1) * XROWS, :])
                for ni in range(2 * RG):
                    half, rl = ni % 2, ni // 2
                    ps = psA.tile([16, 512], F32, name='cps', tag='cps', bufs=4)
                    nc.tensor.matmul(
                        ps[:], W73s[:],
                        xt[0:73, ni * 512:(ni + 1) * 512],
                        start=True, stop=True)
                    ps3 = ps[:].rearrange('p (j t) -> p j t', j=8)
                    dst3 = cg4[:, half * 8:(half + 1) * 8, rl, :]
                    if half == 0:
                        nc.scalar.activation(dst3, ps3, AF.Copy)
                    else:
                        nc.vector.tensor_copy(dst3, ps3)
                # scatter group -> convT r-window, then rms for the window
                for kv in range(2):
                    for g in range(2):
                        for c in range(4):
                            row = g * 8 + kv * 4 + c
                            dst = convT[kv][:].rearrange(
                                '(g c j) (r t) -> g c j r t', g=2, c=4, r=RT)
                            eng = nc.sync if c < 2 else nc.gpsimd
                            eng.dma_start(
                                dst[g, c, :, gi * RG:(gi + 1) * RG, :],
                                cg[row:row + 1, :])
                for kv in range(2):
                    nc.scalar.activation(sqw[kv][:, gsl],
                                         convT[kv][:, gsl], AF.Square)
                    ss2 = psB.tile([128, RG * T], F32, name='ss2', tag='acc')
                    for rl in range(RG):
                        r = gi * RG + rl
                        nc.tensor.matmul(ss2[:, rl * T:(rl + 1) * T], S4s[:],
                                         sqw[kv][:, r * T:(r + 1) * T],
                                         start=True, stop=True)
                    nc.scalar.activation(invP[kv][:, gsl], ss2[:],
                                         AF.Sqrt, bias=epsT[:], scale=0.25)
                    nc.vector.reciprocal(invP[kv][:, gsl], invP[kv][:, gsl])
                    rp = rpk if kv == 0 else rpv
                    nc.vector.scalar_tensor_tensor(
                        nkT[kv][:, gsl], convT[kv][:, gsl], rp[:],
                        invP[kv][:, gsl], OP.mult, OP.mult)
            if debug:
                nc.sync.dma_start(d_nkT[:], nkT[0][:])

            # dense: keys/vals [T, DC] (T on partitions), weights preloaded
            kv_sb = {}
            for kv, (Wsb, bS) in enumerate(((WkS, bkS), (WvS, bvS))):
                ps = psA.tile([T, DC], F32, name='dps', tag='dps', bufs=1)
                for r in range(RT):
                    nc.tensor.matmul(ps[:],
                                     nkT[kv][:, r * T:(r + 1) * T],
                                     Wsb[:, r * DC:(r + 1) * DC],
                                     start=(r == 0), stop=False)
                nc.tensor.matmul(ps[:], o64[:],
                                 bS[:], start=False, stop=True)
                sb = pk.tile([T, DC], BF16, name=f'kv{kv}')
                nc.vector.tensor_copy(sb[:], ps[:])
                kv_sb[kv] = sb
            keys, vals = kv_sb[0], kv_sb[1]
            if debug:
                nc.sync.dma_start(d_keys[:], keys[:])
                nc.sync.dma_start(d_vals[:], vals[:])

            # transpose keys -> keysT chunks [98, 64] x4
            keysT = pk.tile([CQ, NQ * T], BF16, name='keysT')
            for q in range(NQ):
                pt = psT.tile([CQ, T], BF16, name='tps', tag='mmT')
                nc.tensor.transpose(pt[:], keys[:, q * CQ:(q + 1) * CQ],
                                    idnb[0:T, 0:T])
                nc.vector.tensor_copy(keysT[:, q * T:(q + 1) * T], pt[:])

            # scb = bcast(sc), scb2, q2 = vals*scb, scv = scb*vals transposed
            psc = psA.tile([T, DC], F32, name='pscb', tag='dps', bufs=1)
            nc.tensor.matmul(psc[:], o64[:], scS[:],
                             start=True, stop=True)
            scb = pk.tile([T, DC], F32, name='scb')
            nc.vector.tensor_copy(scb[:], psc[:])
            scb2 = pk.tile([T, DC], F32, name='scb2')
            nc.vector.tensor_tensor(scb2[:], scb[:], scb[:], OP.mult)
            q2 = pk.tile([T, DC], BF16, name='q2')
            nc.vector.tensor_tensor(q2[:], vals[:], scb[:], OP.mult)
            P3 = pk.tile([CQ, NQ * 3 * T], BF16, name='P3')
            for q in range(NQ):
                pt = psT.tile([CQ, T], BF16, name='tps', tag='mmT')
                nc.tensor.transpose(pt[:], q2[:, q * CQ:(q + 1) * CQ],
                                    idnb[0:T, 0:T])
                nc.vector.tensor_copy(
                    P3[:, (q * 3 + 1) * T:(q * 3 + 2) * T], pt[:])

            # w2T chunks [98, 512] x4 (PE transposes)
            w2T = pk.tile([CQ, NQ * H], BF16, name='w2T')
            for q in range(NQ):
                for m in range(HT):
                    pt = psT.tile([CQ, 128], BF16, name='t2ps', tag='mmT')
                    nc.tensor.transpose(
                        pt[:], w2S[:, m * DC + q * CQ:
                                   m * DC + (q + 1) * CQ], idnb[:])
                    nc.vector.tensor_copy(
                        w2T[:, q * H + m * 128:q * H + (m + 1) * 128], pt[:])

            # G_K = keys keys^T  (accumulate over chunks)
            pgk = psB.tile([T, T], F32, name='pgk', tag='acc')
            for q in range(NQ):
                nc.tensor.matmul(pgk[:], keysT[:, q * T:(q + 1) * T],
                                 keysT[:, q * T:(q + 1) * T],
                                 start=(q == 0), stop=(q == NQ - 1))
            GK = pk.tile([T, T], BF16, name='GK')
            nc.vector.tensor_copy(GK[:], pgk[:])
            if debug:
                nc.sync.dma_start(d_GK[:], GK[:])

            # z1T partial [H(4x128), T] = w1C^T keysT + b1/8 (one PSUM bank)
            z1ps = psB.tile([128, HT * T], F32, name='z1ps', tag='acc')
            for m in range(HT):
                msl = slice(m * T, (m + 1) * T)
                for q in range(NQ):
                    nc.tensor.matmul(z1ps[:, msl],
                                     w1S[:, q * H + m * 128:
                                         q * H + (m + 1) * 128],
                                     keysT[:, q * T:(q + 1) * T],
                                     start=(q == 0), stop=False)
                nc.tensor.matmul(z1ps[:, msl], b1r8[:, m * 128:(m + 1) * 128],
                                 o64[:], start=False, stop=True)
            z1Tp = pk.tile([128, HT * T], BF16, name='z1Tp')
            nc.vector.tensor_copy(z1Tp[:], z1ps[:])

            # ---- R1: AllReduce [128, 256 z1T cols | 32 GK cols] bf16 ----
            r1i = pd.tile([128, HT * T + 32], BF16, name='r1i')
            r1o = pd.tile([128, HT * T + 32], BF16, name='r1o')
            nc.gpsimd.dma_start(r1i[:, 0:HT * T], z1Tp[:])
            nc.gpsimd.dma_start(
                r1i[:, HT * T:HT * T + 32].rearrange('(p h) c -> p h c',
                                                     h=2),
                GK[:].rearrange('p (h c) -> p h c', h=2))
            nc.gpsimd.collective_compute(
                'AllReduce', OP.add, replica_groups=[list(range(NCORES))],
                ins=[r1i.opt()], outs=[r1o.opt()])

            z1T = pk.tile([128, HT * T], BF16, name='z1T')
            nc.sync.dma_start(z1T[:], r1o[:, 0:HT * T])
            GKg = pk.tile([T, T], BF16, name='GKg')
            nc.sync.dma_start(
                GKg[:].rearrange('p (h c) -> p h c', h=2),
                r1o[:, HT * T:HT * T + 32].rearrange('(p h) c -> p h c',
                                                     h=2))
            if debug:
                for m in range(HT):
                    nc.sync.dma_start(d_z1T[m * 128:(m + 1) * 128, :],
                                      z1T[:, m * T:(m + 1) * T])

            # R64 = diag(wv) @ (GK + 1)  (for z1f correction incl. agg_b1)
            R64 = pk.tile([T, T], BF16, name='R64')
            nc.vector.tensor_scalar(R64[:], GKg[:], 1.0, wvS[:],
                                    OP.add, OP.mult)

            # =========== PHASE 2 ===========
            hT = pk.tile([128, HT * T], BF16, name='hT')
            nc.scalar.activation(hT[:], z1T[:], AF.Gelu_apprx_tanh)
            # h [T, H]
            h = pk.tile([T, H], BF16, name='h')
            for m in range(HT):
                pt = psT.tile([T, 128], BF16, name='hps', tag='mmT')
                nc.tensor.transpose(pt[:], hT[:, m * T:(m + 1) * T], idnb[:])
                nc.vector.tensor_copy(h[:, m * 128:(m + 1) * 128], pt[:])

            # y = h @ w2C  [T, DC]
            py = psA.tile([T, DC], F32, name='py', tag='dps', bufs=1)
            for m in range(HT):
                nc.tensor.matmul(py[:], hT[:, m * T:(m + 1) * T],
                                 w2S[:, m * DC:(m + 1) * DC],
                                 start=(m == 0), stop=(m == HT - 1))
            y = pk.tile([T, DC], BF16, name='y')
            nc.vector.tensor_copy(y[:], py[:])
            if debug:
                nc.sync.dma_start(d_y[:], y[:])

            # yT chunks + (sc^2 y)T into P3 slots (i=2: yT, i=0: s2yT)
            for q in range(NQ):
                ysl = slice((q * 3 + 2) * T, (q * 3 + 3) * T)
                pt = psT.tile([CQ, T], BF16, name='tps', tag='mmT')
                nc.tensor.transpose(pt[:], y[:, q * CQ:(q + 1) * CQ],
                                    idnb[0:T, 0:T])
                nc.vector.tensor_copy(P3[:, ysl], pt[:])
                nc.vector.tensor_scalar(P3[:, (q * 3) * T:(q * 3 + 1) * T],
                                        P3[:, ysl],
                                        scsqTS[:, q:q + 1], None,
                                        OP.mult)

            # scalars C = sum y^2, A = sum (scb y)^2, B = sum (scb y) v
            ua = pk.tile([T, DC], F32, name='ua')
            nc.vector.tensor_tensor(ua[:], y[:], scb[:], OP.mult)
            scr = pk.tile([T, DC], F32, name='scr')
            Cc = pk.tile([T, 1], F32, name='Cc')
            Ac = pk.tile([T, 1], F32, name='Ac')
            Bc = pk.tile([T, 1], F32, name='Bc')
            nc.scalar.activation(scr[:], y[:], AF.Square, accum_out=Cc[:])
            nc.scalar.activation(scr[:], ua[:], AF.Square, accum_out=Ac[:])
            nc.vector.scalar_tensor_tensor(scr[:], ua[:], 1.0, vals[:],
                                           OP.mult, OP.mult,
                                           accum_out=Bc[:])

            # P matmuls: out block (m) = [P1m | P2m | P3m], shared stationary
            Pt = pk.tile([128, 3 * HT * T], BF16, name='Pt')
            for m in range(HT):
                pp = psB.tile([128, 3 * T], F32, name='pp', tag='acc')
                for q in range(NQ):
                    nc.tensor.matmul(
                        pp[:],
                        w2T[:, q * H + m * 128:q * H + (m + 1) * 128],
                        P3[:, q * 3 * T:(q + 1) * 3 * T],
                        start=(q == 0), stop=(q == NQ - 1))
                nc.vector.tensor_copy(
                    Pt[:, m * 3 * T:(m + 1) * 3 * T], pp[:])

            # ---- R2: AllReduce [128, 768 P cols | 3 C/A/B cols] bf16 ----
            NP2 = 3 * HT * T
            r2i = pd.tile([128, NP2 + 3], BF16, name='r2i')
            r2o = pd.tile([128, NP2 + 3], BF16, name='r2o')
            nc.gpsimd.dma_start(r2i[:, 0:NP2], Pt[:])
            nc.gpsimd.dma_start(r2i[0:T, NP2 + 0:NP2 + 1], Cc[:])
            nc.gpsimd.dma_start(r2i[0:T, NP2 + 1:NP2 + 2], Ac[:])
            nc.gpsimd.dma_start(r2i[0:T, NP2 + 2:NP2 + 3], Bc[:])
            nc.gpsimd.collective_compute(
                'AllReduce', OP.add, replica_groups=[list(range(NCORES))],
                ins=[r2i.opt()], outs=[r2o.opt()])

            Pg = pk.tile([128, 3 * HT * T], BF16, name='Pg')
            nc.sync.dma_start(Pg[:], r2o[:, 0:NP2])
            CAB = pk.tile([T, 3], BF16, name='CAB')
            nc.sync.dma_start(CAB[:], r2o[0:T, NP2:NP2 + 3])

            # scalar chain, column space [T, 1] (cols of scol: inv a1 a2 a3)
            scol = pk.tile([T, 4], F32, name='scol')
            i2c = pk.tile([T, 1], F32, name='i2c')
            t1c = pk.tile([T, 1], F32, name='t1c')
            Scc = pk.tile([T, 1], F32, name='Scc')
            nc.scalar.activation(scol[:, 0:1], CAB[:, 0:1], AF.Sqrt,
                                 bias=epsT[0:T, :], scale=1.0 / D)
            nc.vector.reciprocal(scol[:, 0:1], scol[:, 0:1])
            nc.vector.tensor_tensor(i2c[:], scol[:, 0:1], scol[:, 0:1],
                                    OP.mult)
            # S = 2 inv A - 2 B
            nc.vector.scalar_tensor_tensor(t1c[:], scol[:, 0:1], 2.0,
                                           CAB[:, 1:2], OP.mult, OP.mult)
            nc.vector.scalar_tensor_tensor(Scc[:], CAB[:, 2:3], -2.0,
                                           t1c[:], OP.mult, OP.add)
            # a1 = 2 inv^2 ; a2 = 2 inv ; a3 = inv^3 S / D
            nc.vector.tensor_scalar(scol[:, 1:2], i2c[:], 2.0, None, OP.mult)
            nc.vector.tensor_scalar(scol[:, 2:3], scol[:, 0:1], 2.0, None,
                                    OP.mult)
            nc.vector.scalar_tensor_tensor(t1c[:], i2c[:], 1.0 / D,
                                           scol[:, 0:1], OP.mult, OP.mult)
            nc.vector.tensor_tensor(scol[:, 3:4], t1c[:], Scc[:], OP.mult)

            # a1/a2/a3 to a single [1, 3T] row, then broadcast ab2 [128, 3T]
            r3ps = psB.tile([1, 3 * T], F32, name='r3ps', tag='acc')
            for j in range(3):
                nc.tensor.matmul(r3ps[:, j * T:(j + 1) * T],
                                 scol[:, 1 + j:2 + j], idn[0:T, 0:T],
                                 start=True, stop=True)
            arow = pk.tile([1, 3 * T], F32, name='arow')
            nc.vector.tensor_copy(arow[:], r3ps[:])
            abps = psB.tile([128, 3 * T], F32, name='abps', tag='acc')
            for j in range(3):
                nc.tensor.matmul(abps[:, j * T:(j + 1) * T], o128[:],
                                 arow[:, j * T:(j + 1) * T],
                                 start=True, stop=True)
            ab2 = pk.tile([128, 3 * T], F32, name='ab2')
            nc.vector.tensor_copy(ab2[:], abps[:])

            # dhT = a1*P1 - a2*P2 - a3*P3 ; dz1T = dhT * gelu'(z1T)
            dgel = pk.tile([128, HT * T], BF16, name='dgel')
            nc.scalar.activation(dgel[:], z1T[:], AF.Derivative_Gelu)
            dhT = pk.tile([128, HT * T], F32, name='dhT')
            tmpA = pk.tile([128, 3 * T], F32, name='tmpA')
            for m in range(HT):
                msl = slice(m * T, (m + 1) * T)
                nc.vector.tensor_tensor(tmpA[:],
                                        Pg[:, m * 3 * T:(m + 1) * 3 * T],
                                        ab2[:], OP.mult)
                nc.vector.tensor_tensor(dhT[:, msl], tmpA[:, 0:T],
                                        tmpA[:, T:2 * T], OP.subtract)
                nc.vector.tensor_tensor(dhT[:, msl], dhT[:, msl],
                                        tmpA[:, 2 * T:3 * T], OP.subtract)
            dz1T = pk.tile([128, HT * T], BF16, name='dz1T')
            nc.vector.tensor_tensor(dz1T[:], dhT[:], dgel[:], OP.mult)

            # dz1 [T, H]
            dz1 = pk.tile([T, H], BF16, name='dz1')
            for m in range(HT):
                pt = psT.tile([T, 128], BF16, name='dzps', tag='mmT')
                nc.tensor.transpose(pt[:], dz1T[:, m * T:(m + 1) * T], idnb[:])
                nc.vector.tensor_copy(dz1[:, m * 128:(m + 1) * 128], pt[:])

            # z1fT = z1T - dz1^T-weighted: T2T[m] = dz1[:,m]^T @ R64
            t2ps = psB.tile([128, HT * T], F32, name='t2t', tag='acc')
            for m in range(HT):
                nc.tensor.matmul(t2ps[:, m * T:(m + 1) * T],
                                 dz1[:, m * 128:(m + 1) * 128],
                                 R64[:], start=True, stop=True)
            z1fT = pk.tile([128, HT * T], F32, name='z1fT')
            nc.vector.tensor_tensor(z1fT[:], z1T[:], t2ps[:], OP.subtract)
            hfT = pk.tile([128, HT * T], BF16, name='hfT')
            nc.scalar.activation(hfT[:], z1fT[:], AF.Gelu_apprx_tanh)

            # G = a1*(scb2*y) - a2*(q2) - a3*y  (column scalars)
            G = pk.tile([T, DC], F32, name='G')
            gt1 = pk.tile([T, DC], F32, name='gt1')
            nc.vector.tensor_tensor(gt1[:], y[:], scb2[:], OP.mult)
            nc.vector.tensor_scalar(G[:], gt1[:], scol[:, 1:2], None, OP.mult)
            nc.vector.tensor_scalar(gt1[:], q2[:], scol[:, 2:3], None, OP.mult)
            nc.vector.tensor_tensor(G[:], G[:], gt1[:], OP.subtract)
            nc.vector.tensor_scalar(gt1[:], y[:], scol[:, 3:4], None, OP.mult)
            nc.vector.tensor_tensor(G[:], G[:], gt1[:], OP.subtract)

            # agg_w2 & w2' = w2 - h^T (wv*G)
            wG = pk.tile([T, DC], BF16, name='wG')
            nc.vector.tensor_scalar(wG[:], G[:], wvS[:], None, OP.mult)
            w2p = pk.tile([128, HT * DC], BF16, name='w2p')
            for m in range(HT):
                pa = psA.tile([128, DC], F32, name='paw2', tag='dps', bufs=1)
                nc.tensor.matmul(pa[:],
                                 h[:, m * 128:(m + 1) * 128],
                                 wG[:], start=True, stop=True)
                nc.vector.tensor_tensor(w2p[:, m * DC:(m + 1) * DC],
                                        w2S[:, m * DC:(m + 1) * DC], pa[:],
                                        OP.subtract)
                if debug:
                    nc.sync.dma_start(d_w2p[m * 128:(m + 1) * 128, :],
                                      w2p[:, m * DC:(m + 1) * DC])

            # rows: b2' ; sc' ; sc'*ros (all partition-0 tiles)
            brow = pk.tile([1, 3 * DC], F32, name='brow')
            pr = psB.tile([1, DC], F32, name='prow', tag='acc')
            nc.tensor.matmul(pr[:], wvS[:], G[:],
                             start=True, stop=True)
            nc.vector.tensor_tensor(brow[:, 0:DC], b2S[:], pr[:], OP.subtract)

            # r2y = 2*inv*(scb*y)*y - 2*v*y ; agg_sc = (wv*inv)^T r2y
            nc.vector.tensor_tensor(gt1[:], ua[:], y[:], OP.mult)
            nc.vector.tensor_scalar(gt1[:], gt1[:], scol[:, 2:3], None, OP.mult)
            r2y2 = pk.tile([T, DC], F32, name='r2y2')
            nc.vector.tensor_tensor(r2y2[:], vals[:], y[:], OP.mult)
            nc.vector.tensor_scalar(r2y2[:], r2y2[:], 2.0, None, OP.mult)
            nc.vector.tensor_tensor(gt1[:], gt1[:], r2y2[:], OP.subtract)
            wiv = pk.tile([T, 1], F32, name='wiv')
            nc.vector.tensor_tensor(wiv[:], wvS[:], scol[:, 0:1], OP.mult)
            pr2 = psB.tile([1, DC], F32, name='prow2', tag='acc')
            nc.tensor.matmul(pr2[:], wiv[:],
                             gt1[:], start=True, stop=True)
            nc.vector.tensor_tensor(brow[:, DC:2 * DC], scS32[:], pr2[:],
                                    OP.subtract)
            nc.vector.tensor_tensor(brow[:, 2 * DC:3 * DC],
                                    brow[:, DC:2 * DC], rosS[:], OP.mult)

            # transpose rows to columns: colrows[:, q*3+j]
            colrows = pk.tile([CQ, NQ * 3], F32, name='colrows')
            for q in range(NQ):
                pt = psB.tile([CQ, 3], F32, name='crps', tag='acc')
                for j in range(3):
                    nc.tensor.transpose(
                        pt[:, j:j + 1],
                        brow[:, j * DC + q * CQ:j * DC + (q + 1) * CQ],
                        idn[0:1, 0:1])
                nc.vector.tensor_copy(colrows[:, q * 3:(q + 1) * 3], pt[:])

            # yfT chunks [98, T] = w2p^T @ hfT + b2'T ; squares and partials
            yfT = pk.tile([CQ, NQ * T], F32, name='yfT')
            sqf = pk.tile([CQ, NQ * T], BF16, name='sqf')
            ssqf = pk.tile([CQ, NQ * T], BF16, name='ssqf')
            for q in range(NQ):
                pf = psB.tile([CQ, T], F32, name='pyf', tag='acc')
                for m in range(HT):
                    nc.tensor.matmul(pf[:],
                                     w2p[:, m * DC + q * CQ:m * DC + (q + 1) * CQ],
                                     hfT[:, m * T:(m + 1) * T],
                                     start=(m == 0), stop=(m == HT - 1))
                sl = slice(q * T, (q + 1) * T)
                nc.vector.tensor_scalar(yfT[:, sl], pf[:],
                                        colrows[:, q * 3:q * 3 + 1], None,
                                        OP.add)
                nc.vector.tensor_tensor(sqf[:, sl], yfT[:, sl], yfT[:, sl],
                                        OP.mult)
                nc.vector.tensor_scalar(ssqf[:, sl], yfT[:, sl],
                                        colrows[:, q * 3 + 1:q * 3 + 2], None,
                                        OP.mult)
                nc.vector.tensor_tensor(ssqf[:, sl], ssqf[:, sl], ssqf[:, sl],
                                        OP.mult)
            if debug:
                for q in range(NQ):
                    nc.sync.dma_start(d_yfT[q * CQ:(q + 1) * CQ, :],
                                      yfT[:, q * T:(q + 1) * T])
            pfin = psB.tile([1, 2 * T], F32, name='pfin', tag='acc')
            for q in range(NQ):
                nc.tensor.matmul(pfin[:, 0:T], ocol[0:CQ, :],
                                 sqf[:, q * T:(q + 1) * T],
                                 start=(q == 0), stop=(q == NQ - 1))
            for q in range(NQ):
                nc.tensor.matmul(pfin[:, T:2 * T], ocol[0:CQ, :],
                                 ssqf[:, q * T:(q + 1) * T],
                                 start=(q == 0), stop=(q == NQ - 1))
            fin = pk.tile([1, 2 * T], F32, name='fin')
            nc.vector.tensor_copy(fin[:], pfin[:])

            # ---- R3: AllReduce final scalars ----
            r3i = pd.tile([1, 2 * T], F32, name='r3i')
            r3o = pd.tile([1, 2 * T], F32, name='r3o')
            nc.gpsimd.dma_start(r3i[:], fin[:])
            nc.gpsimd.collective_compute(
                'AllReduce', OP.add, replica_groups=[list(range(NCORES))],
                ins=[r3i.opt()], outs=[r3o.opt()])

            # invf = rsqrt(Cf/D + eps); invp = rsqrt(invf^2 * Af/D + eps)
            CfAf = pk.tile([1, 2 * T], F32, name='CfAf')
            nc.sync.dma_start(CfAf[:], r3o[:])
            invft = pk.tile([1, T], F32, name='invft')
            invpt = pk.tile([1, T], F32, name='invpt')
            fft = pk.tile([1, T], F32, name='fft')
            nc.scalar.activation(invft[:], CfAf[:, 0:T], AF.Sqrt,
                                 bias=epsT[0:1, :], scale=1.0 / D)
            nc.vector.reciprocal(invft[:], invft[:])
            nc.vector.tensor_tensor(invpt[:], invft[:], invft[:], OP.mult)
            nc.vector.tensor_tensor(invpt[:], invpt[:], CfAf[:, T:2 * T],
                                    OP.mult)
            nc.scalar.activation(invpt[:], invpt[:], AF.Sqrt,
                                 bias=epsT[0:1, :], scale=1.0 / D)
            nc.vector.reciprocal(invpt[:], invpt[:])
            nc.vector.tensor_tensor(fft[:], invft[:], invpt[:], OP.mult)
            ffb = pk.tile([128, T], F32, name='ffb')
            pt = psB.tile([128, T], F32, name='ffps', tag='acc')
            nc.tensor.matmul(pt[:], o128[:], fft[:], start=True, stop=True)
            nc.vector.tensor_copy(ffb[:], pt[:])

            # out = yfT * scrosT * ff  (single [CQ, NQ*T] store)
            outsb = pk.tile([CQ, NQ * T], F32, name='outsb')
            for q in range(NQ):
                sl = slice(q * T, (q + 1) * T)
                nc.vector.scalar_tensor_tensor(
                    outsb[:, sl], yfT[:, sl],
                    colrows[:, q * 3 + 2:q * 3 + 3], ffb[0:CQ, :],
                    OP.mult, OP.mult)
            nc.sync.dma_start(out[:], outsb[:])

    _split_excess_waits(nc)
    return nc, sorted(dbg_outs.keys())


def make_inputs(inputs):
    """Build the 8 per-core input dicts from the full problem inputs."""
    x = np.asarray(inputs['x'], np.float32)
    x_t = np.transpose(x, (0, 2, 3, 1))
    X72 = _build_im2col(x_t)
    W73 = _build_w73(np.asarray(inputs['conv_k_w'], np.float32),
                     np.asarray(inputs['conv_k_b'], np.float32),
                     np.asarray(inputs['conv_v_w'], np.float32),
                     np.asarray(inputs['conv_v_b'], np.float32))
    perm, valid = _din_perm()
    dkw = np.asarray(inputs['dense_k_w'], np.float32)
    dvw = np.asarray(inputs['dense_v_w'], np.float32)
    Wk_full = np.zeros((DINP, D), np.float32)
    Wv_full = np.zeros((DINP, D), np.float32)
    Wk_full[valid] = dkw[perm[valid]]
    Wv_full[valid] = dvw[perm[valid]]

    w1 = np.asarray(inputs['mem_w1'], np.float32)
    w2 = np.asarray(inputs['mem_w2'], np.float32)
    sc = np.asarray(inputs['mem_scale'], np.float32)
    ros = np.asarray(inputs['rms_out_scale'], np.float32)
    dkb = np.asarray(inputs['dense_k_b'], np.float32)
    dvb = np.asarray(inputs['dense_v_b'], np.float32)
    b1 = np.asarray(inputs['mem_b1'], np.float32)
    b2 = np.asarray(inputs['mem_b2'], np.float32)

    base = {
        'X80': _pack_slabs(X72).astype(NPBF),
    }
    CE = np.zeros((128, 144), np.float32)
    CE[:, 0:128] = _s4()
    CE[0:73, 128:144] = W73
    base['CE'] = CE.astype(NPBF)
    CF = np.zeros((128, 2), np.float32)
    CF[:, 0:1] = _rms_pattern(np.asarray(inputs['rms_k_scale'], np.float32))
    CF[:, 1:2] = _rms_pattern(np.asarray(inputs['rms_v_scale'], np.float32))
    base['CF'] = CF

    in_maps = []
    for c in range(NCORES):
        sl = slice(c * DC, (c + 1) * DC)
        m = dict(base)
        CA = np.zeros(3 * DC + T + H, np.float32)
        CA[0:DC] = dkb[sl]
        CA[DC:2 * DC] = dvb[sl]
        CA[2 * DC:3 * DC] = sc[sl]
        CA[3 * DC:3 * DC + T] = 1.0
        CA[3 * DC + T:] = b1 / NCORES
        m['CA'] = CA.reshape(1, -1).astype(NPBF)
        CB = np.zeros(3 * DC + 128, np.float32)
        CB[0:DC] = b2[sl]
        CB[DC:2 * DC] = sc[sl]
        CB[2 * DC:3 * DC] = ros[sl]
        CB[3 * DC:] = 1.0
        m['CB'] = CB.reshape(1, -1)
        CD = np.zeros((128, 133), np.float32)
        CD[:, 0:128] = np.eye(128)
        CD[0:CQ, 128:132] = (sc[sl] ** 2).reshape(NQ, CQ).T
        CD[0:T, 132] = _wvec()
        m['CD'] = CD
        CC = np.zeros((128, 129 + HT * DC + NQ * H + 2 * RT * DC),
                      np.float32)
        CC[:, 0:128] = np.eye(128)
        CC[:, 128] = 1.0
        o = 129
        w2c = w2[:, sl]
        CC[:, o:o + HT * DC] = (w2c.reshape(HT, 128, DC).transpose(1, 0, 2)
                                .reshape(128, HT * DC))
        o += HT * DC
        w1c = w1[sl, :]
        CC[0:CQ, o:o + NQ * H] = (w1c.reshape(NQ, CQ, H).transpose(1, 0, 2)
                                  .reshape(CQ, NQ * H))
        o += NQ * H
        CC[:, o:o + RT * DC] = (Wk_full[:, sl].reshape(RT, 128, DC)
                                .transpose(1, 0, 2).reshape(128, RT * DC))
        o += RT * DC
        CC[:, o:o + RT * DC] = (Wv_full[:, sl].reshape(RT, 128, DC)
                                .transpose(1, 0, 2).reshape(128, RT * DC))
        m['CC'] = CC.astype(NPBF)
        in_maps.append(m)
    return in_maps


def kernel(**inputs):
    if 'nc' not in _NC_CACHE:
        _NC_CACHE['nc'], _ = build_nc(debug=False)
    nc = _NC_CACHE['nc']
    in_maps = make_inputs(inputs)
    res = run_bass_kernel_spmd(nc, in_maps, list(range(NCORES)))
    blocks = [res.results[c]['out'].reshape(CQ, NQ, T).transpose(1, 0, 2)
              .reshape(DC, T) for c in range(NCORES)]
    YT = np.concatenate(blocks, axis=0)
    return np.ascontiguousarray(YT.T).reshape(T, 4, 28, 28)

